# revision 27
# baseline (speedup 1.0000x reference)
"""Trainium2 Bass kernel for nn_DatacubeLLMBridge (dense_transformer).

Sharding: 8 cores = 4 batches x 2 sequence-halves. Core c owns batch c//2,
rows [(c%2)*512, (c%2+1)*512) of S=1024. Weights replicated. Cross-core
traffic: AllGather of K/V within pairs [[0,1],[2,3],[4,5],[6,7]] (4 total).

Layout: activations are FEATURE-major in SBUF: x^T = [feat chunks of 128
partitions, rows in free dim]. Matmuls contract the partition dim. Weights
host-pretransposed to W^T=[fin,fout] bf16. Row-major V (for attention AV)
is produced by swapping matmul operands (lhsT = x^T row-chunk).

Softmax: the reference's energy/mass biases are constant per query row and
softmax is shift-invariant per row, so they cancel exactly. Score
magnitudes are O(1) here (0.02-scale weights), so exp without
max-subtraction is safe. Softmax denominators come from a ones-augmented V
column; the normalization is folded into the AV-PSUM evict.

PSUM budget (8 banks): dense [128,512]x2 + scores [128,4,512]x1 + avt
[65,512]x2 = 8.
"""

from contextlib import ExitStack

import numpy as np
import ml_dtypes

import concourse.bass as bass
import concourse.mybir as mybir
import concourse.tile as tile
from concourse import bacc
from concourse.bass_utils import run_bass_kernel_spmd

F32 = mybir.dt.float32
BF16 = mybir.dt.bfloat16
AF = mybir.ActivationFunctionType
OP = mybir.AluOpType


# Unify Exp/Ln into one ACT table set (natural_log_exp_and_others) so the
# softmax-reciprocal ln/exp does not thrash ACT_TABLE_LOADs. Contents-only
# edit; set order (act_func_set_id indices) is preserved.
import functools as _ft
import concourse.hw_specs as _hw
import concourse.bacc as _bacc_mod

_ORIG_GAT = _hw.get_activation_tables


@_ft.cache
def _patched_gat(arch):
    t = dict(_ORIG_GAT(arch))
    drop = {AF.Exp, AF.Ln}
    for name in ("exp_and_others", "natural_log"):
        if name in t:
            t[name] = set(t[name]) - drop
    return t


_hw.get_activation_tables = _patched_gat
_bacc_mod.get_activation_tables = _patched_gat

P = 128
B, S, CNN_D, LLM_D, BR_D, H = 4, 1024, 512, 768, 1024, 16
R = 512            # rows per core
N_CORES = 8
GROUPS = [[0, 1], [2, 3], [4, 5], [6, 7]]
EPS = 1e-5
NK = S + 3         # phys attention keys

bf16 = ml_dtypes.bfloat16


class Ker:
    """Holds build state so helpers can share pools/constants."""

    def __init__(self, nc, tc, d):
        self.nc, self.tc, self.d = nc, tc, d
        self.rr_recip = True

    # ------------------------------------------------------------ helpers
    def linear_fm(self, out_sb, wT, x_sb, bias, kch, mch, func=AF.Copy,
                  wcol0=0, uid=""):
        """out^T[:,m,:] = func(sum_k wT[:,k,col].T @ x[:,k,:] + bias[:,m])."""
        nc = self.nc
        ncols = x_sb.shape[-1]
        for m in range(mch):
            ps = self.pps.tile([P, 512], F32, tag="dense", bufs=2,
                               name=f"psd_{uid}_{m}")
            for k in range(kch):
                nc.tensor.matmul(
                    ps[:, :ncols],
                    lhsT=wT[:, k, wcol0 + 128 * m: wcol0 + 128 * (m + 1)],
                    rhs=x_sb[:, k, :],
                    start=(k == 0), stop=(k == kch - 1))
            if func == AF.Copy:
                nc.vector.tensor_scalar(out_sb[:, m, :], ps[:, :ncols],
                                        scalar1=bias[:, m:m + 1],
                                        scalar2=None, op0=OP.add)
            else:
                nc.scalar.activation(out_sb[:, m, :], ps[:, :ncols], func,
                                     bias=bias[:, m:m + 1])

    def vproj_rm(self, out_sb, wT, x_sb, bias_rep, kch, rch, fout,
                 wcol0=0, uid=""):
        """Row-major projection: out[:,rc,:] = x-rows @ W^T + bias."""
        nc = self.nc
        for rc in range(rch):
            for nn in range((fout + 511) // 512):
                n0, n1 = nn * 512, min(fout, (nn + 1) * 512)
                ps = self.pps.tile([P, 512], F32, tag="dense", bufs=2,
                                   name=f"psv_{uid}_{rc}_{nn}")
                for k in range(kch):
                    nc.tensor.matmul(
                        ps[:, : n1 - n0],
                        lhsT=x_sb[:, k, 128 * rc: 128 * (rc + 1)],
                        rhs=wT[:, k, wcol0 + n0: wcol0 + n1],
                        start=(k == 0), stop=(k == kch - 1))
                nc.vector.tensor_add(out_sb[:, rc, n0:n1], ps[:, : n1 - n0],
                                     bias_rep[:, n0:n1])

    def layernorm_fm(self, out_sb, in_sb, gamma, beta, mch, D, uid,
                     out_fn=None):
        """LN over the feature (partition-chunk) dim; in_sb [128,mch,512].
        If out_fn is given it must return the destination AP for chunk m."""
        nc, sp = self.nc, self.spln
        sq = sp.tile([P, 8, 512], BF16, tag="ln_sq", name=f"lnsq_{uid}")
        nc.vector.tensor_mul(sq[:, :mch, :], in_sb[:, :mch, :],
                             in_sb[:, :mch, :])
        ps_s = self.pps.tile([1, 512], F32, tag="dense", bufs=2,
                             name=f"lns_{uid}")
        ps_q = self.pps.tile([1, 512], F32, tag="dense", bufs=2,
                             name=f"lnq_{uid}")
        for k in range(mch):
            nc.tensor.matmul(ps_s[:], lhsT=self.ones_bf[:, 0:1],
                             rhs=in_sb[:, k, :],
                             start=(k == 0), stop=(k == mch - 1))
        for k in range(mch):
            nc.tensor.matmul(ps_q[:], lhsT=self.ones_bf[:, 0:1],
                             rhs=sq[:, k, :],
                             start=(k == 0), stop=(k == mch - 1))
        sm = sp.tile([1, 2, 512], F32, tag="ln_sm", name=f"lnsm_{uid}")
        mean, msq = sm[0:1, 0, :], sm[0:1, 1, :]
        rstd = sp.tile([1, 512], F32, tag="ln_rstd", name=f"lnrs_{uid}")
        mr = sp.tile([1, 512], F32, tag="ln_mr", name=f"lnmr_{uid}")
        nc.vector.tensor_scalar_mul(mean, ps_s[:], 1.0 / D)
        nc.vector.tensor_scalar_mul(msq, ps_q[:], 1.0 / D)
        nc.vector.tensor_mul(rstd[:], mean, mean)      # rstd = mean^2 (tmp)
        nc.vector.tensor_sub(msq, msq, rstd[:])        # msq  = var
        nc.scalar.activation(msq, msq, AF.Sqrt, bias=self.eps_t[0:1, :])
        nc.vector.reciprocal(rstd[:], msq)
        nc.vector.tensor_mul(mr[:], mean, rstd[:])
        bc = sp.tile([P, 2, 512], BF16, tag="ln_bc", name=f"lnbc_{uid}")
        for j, src in ((0, rstd[:]), (1, mr[:])):
            ps_b = self.pps.tile([P, 512], F32, tag="dense", bufs=2,
                                 name=f"lnb_{uid}_{j}")
            nc.tensor.matmul(ps_b[:], lhsT=self.ones_1x128[:], rhs=src,
                             start=True, stop=True)
            nc.vector.tensor_copy(bc[:, j, :], ps_b[:])
        for m in range(mch):
            t = sp.tile([P, 512], BF16, tag="ln_t", bufs=1,
                        name=f"lnt_{uid}_{m}")
            nc.vector.tensor_mul(t[:], in_sb[:, m, :], bc[:, 0, :])
            nc.vector.tensor_sub(t[:], t[:], bc[:, 1, :])
            dst = out_fn(m) if out_fn is not None else out_sb[:, m, :]
            nc.vector.tensor_scalar(dst, t[:],
                                    scalar1=gamma[:, m:m + 1],
                                    scalar2=beta[:, m:m + 1],
                                    op0=OP.mult, op1=OP.add)

    def attention(self, qT, kT_full, v_packed, attnoutT, dh, n_keys, uid):
        """qT [128,E/128,512]; kT_full [128,E/128,n_keys];
        v_packed [128,kcN,H,dh+1] (ones at col dh). Writes normalized
        attnoutT [128,E/128,512] bf16. dh in {32, 64}.

        Per (group, pair): the kc loop interleaves scores -> exp -> AV so
        the exp tiles are a small ring (SBUF) and ACT stays saturated.
        Softmax reciprocal alternates DVE <-> ACT(ln/exp) to balance
        engine load (ACT table sets unified via get_activation_tables
        patch)."""
        nc, sp = self.nc, self.spat
        E = qT.shape[1] * P
        hpc = P // dh
        kc_full, rag = n_keys // P, n_keys % P
        kcN = kc_full + (1 if rag else 0)
        scale = 1.0 / float(np.sqrt(dh))
        npair = hpc // 2
        for g in range(E // P):
            for a in range(npair):
                ps_avs = []
                rps = []
                for i in range(2):
                    ps_avs.append(self.pps.tile(
                        [65, 512], F32, tag="avt", bufs=4,
                        name=f"av_{uid}_{g}_{a}_{i}"))
                for kc in range(kcN):
                    kk = P if kc < kc_full else rag
                    ps_s = self.pps.tile([P, 2, 512], F32, tag="sc", bufs=1,
                                         name=f"sc_{uid}_{g}_{kc}_{a}")
                    for i in range(2):
                        ho = (2 * a + i) * dh
                        nc.tensor.matmul(
                            ps_s[:kk, i, :],
                            lhsT=kT_full[ho:ho + dh, g, kc * P: kc * P + kk],
                            rhs=qT[ho:ho + dh, g, :],
                            start=True, stop=True,
                            tile_position=(ho, 0))
                    e = sp.tile([P, 2, 512], BF16, tag="expS", bufs=3,
                                name=f"exp_{uid}_{g}_{a}_{kc}")
                    nc.scalar.activation(e[:kk, :, :], ps_s[:kk, :, :],
                                         AF.Exp, scale=scale)
                    h0 = g * hpc + 2 * a
                    for i in range(2):
                        nc.tensor.matmul(
                            ps_avs[i][: dh + 1, :],
                            lhsT=v_packed[:kk, kc, h0 + i, :],
                            rhs=e[:kk, i, :],
                            start=(kc == 0), stop=(kc == kcN - 1))
                for i in range(2):
                    rp = sp.tile([1, 512], F32, tag="recip", bufs=2,
                                 name=f"rp_{uid}_{g}_{a}_{i}")
                    if self.rr_recip:
                        nc.vector.reciprocal(rp[:], ps_avs[i][dh:dh + 1, :])
                    else:
                        nc.scalar.activation(rp[:], ps_avs[i][dh:dh + 1, :],
                                             AF.Ln)
                        nc.scalar.activation(rp[:], rp[:], AF.Exp,
                                             scale=-1.0)
                    self.rr_recip = not self.rr_recip
                    rps.append(rp)
                ps_bc = self.pps.tile([P, 512], F32, tag="dense", bufs=2,
                                      name=f"bc_{uid}_{g}_{a}")
                for i in range(2):
                    ho = (2 * a + i) * dh
                    nc.tensor.matmul(ps_bc[ho:ho + dh, :],
                                     lhsT=self.ones_1x128[0:1, 0:dh],
                                     rhs=rps[i][:],
                                     start=True, stop=True,
                                     tile_position=(0, ho))
                bo0 = 2 * a * dh
                bc_sb = sp.tile([P, 512], BF16, tag="bc_sb", bufs=2,
                                name=f"bcs_{uid}_{g}_{a}")
                nc.scalar.activation(bc_sb[bo0:bo0 + 2 * dh, :],
                                     ps_bc[bo0:bo0 + 2 * dh, :], AF.Copy)
                for i in range(2):
                    ho = (2 * a + i) * dh
                    nc.vector.tensor_tensor(
                        attnoutT[ho:ho + dh, g, :], ps_avs[i][0:dh, :],
                        bc_sb[ho:ho + dh, :], op=OP.mult)


def _build_body(K, ctx):
    nc, tc, d = K.nc, K.tc, K.d

    # ---------------- persistent pools
    pc = ctx.enter_context(tc.tile_pool(name="consts", bufs=1))
    st = ctx.enter_context(tc.tile_pool(name="stream", bufs=1))
    dram = ctx.enter_context(tc.tile_pool(name="drampool", bufs=1,
                                          space="DRAM"))
    K.pps = ctx.enter_context(tc.tile_pool(name="pspool", bufs=1,
                                           space="PSUM"))
    K.spln = ctx.enter_context(tc.tile_pool(name="sp_ln", bufs=1))
    K.spat = ctx.enter_context(tc.tile_pool(name="sp_att", bufs=1))
    spq = ctx.enter_context(tc.tile_pool(name="sp_q", bufs=1))

    def load_pp(name, cols, dt=F32):
        t = pc.tile([P, cols], dt, name=f"c_{name}")
        nc.sync.dma_start(t[:], d[name][:])
        return t

    bias = {n: load_pp(n, c) for n, c in [
        ("b_saq", 4), ("b_sak", 4), ("b_sao", 4), ("b_taq", 4), ("b_tak", 4),
        ("b_tao", 4), ("b_c2b", 8), ("b_l2b", 8), ("b_pq", 8), ("b_pk", 8),
        ("b_po", 8), ("b_c2l", 6), ("b_l2c", 4), ("g_c2b", 8), ("e_c2b", 8),
        ("g_l2b", 8), ("e_l2b", 8), ("g_pa", 8), ("e_pa", 8), ("g_c2l", 6),
        ("e_c2l", 6), ("g_l2c", 4), ("e_l2c", 4)]}
    for n, c in [("b_sav", CNN_D), ("b_tav", CNN_D), ("b_pv", BR_D)]:
        bias[n] = load_pp(n, c, dt=BF16)

    K.ones_bf = pc.tile([P, 1], BF16, name="ones_bf")
    nc.vector.memset(K.ones_bf[:], 1.0)
    K.ones_1x128 = pc.tile([1, P], F32, name="ones_1x128")
    nc.vector.memset(K.ones_1x128[:], 1.0)
    K.ones_1x128_bf = pc.tile([1, P], BF16, name="ones_1x128_bf")
    nc.vector.memset(K.ones_1x128_bf[:], 1.0)
    K.eps_t = pc.tile([1, 1], F32, name="eps_t")
    nc.vector.memset(K.eps_t[:], EPS)
    physT = pc.tile([P, 8, 3], BF16, name="physT_sb")
    nc.sync.dma_start(physT[:], d["physT"].rearrange("(c p) t -> p c t", p=P))

    def load_w(pool, name, kch, fout, tag=None, bufs=1):
        t = pool.tile([P, kch, fout], BF16, tag=tag or f"w{name}", bufs=bufs,
                      name=f"w_{name}")
        nc.sync.dma_start(t[:],
                          d[f"w_{name}"].rearrange("(c p) n -> p c n", p=P))
        return t

    aug_l = st.tile([P, 8, R], BF16, tag="aug", bufs=2, name="aug_l")
    aug_c = st.tile([P, 8, R], BF16, tag="aug", bufs=2, name="aug_c")
    q_c = spq.tile([P, 8, R], BF16, tag="physq", bufs=2, name="q_c")
    q_l = spq.tile([P, 8, R], BF16, tag="physq", bufs=2, name="q_l")
    kT_phys = spq.tile([P, 8, 3], BF16, name="kT_phys")
    v_phys = spq.tile([3, BR_D], BF16, name="v_phys")

    # ======================================================= front stages
    with tc.tile_pool(name="front", bufs=1) as front, \
         tc.tile_pool(name="w_c", bufs=1) as wC, \
         tc.tile_pool(name="sp_br", bufs=1) as spbr:

        x0 = front.tile([P, 4, R], BF16, tag="xs", bufs=2, name="x0")
        nc.sync.dma_start(x0[:], d["xc"].rearrange("(c p) j -> p c j", p=P))
        xl = front.tile([P, 6, R], BF16, name="xl")
        nc.sync.dma_start(xl[:], d["xl"].rearrange("(c p) j -> p c j", p=P))

        def bridge(x_sb, wt, bname, mch, D, out_sb, uid, kch):
            g = spbr.tile([P, 8, R], BF16, tag="gelu", bufs=1,
                          name=f"gelu_{uid}")
            K.linear_fm(g, wt, x_sb, bias[f"b_{bname}"], kch, mch,
                        func=AF.Gelu, uid=f"br_{uid}")
            K.layernorm_fm(out_sb, g, bias[f"g_{bname}"], bias[f"e_{bname}"],
                           mch, D, uid)

        with tc.tile_pool(name="w_ab", bufs=1) as wAB, \
             tc.tile_pool(name="sp_mha", bufs=1) as spm, \
             tc.tile_pool(name="w_d", bufs=1) as wD, \
             tc.tile_pool(name="sp_d", bufs=1) as spD:

            def mha_front(x_sb, wqkv, bq, bk, bv_rep, mid):
                qT = spm.tile([P, 4, R], BF16, tag="qT", name=f"qT_{mid}")
                kTl = spm.tile([P, 4, R], BF16, tag="kTl", name=f"kTl_{mid}")
                vl = spm.tile([P, 4, CNN_D], BF16, tag="vl", name=f"vl_{mid}")
                K.linear_fm(qT, wqkv, x_sb, bq, 4, 4, wcol0=0, uid=f"q{mid}")
                K.linear_fm(kTl, wqkv, x_sb, bk, 4, 4, wcol0=512,
                            uid=f"k{mid}")
                K.vproj_rm(vl, wqkv, x_sb, bv_rep, 4, 4, CNN_D, wcol0=1024,
                           uid=f"v{mid}")
                kv_loc = dram.tile([2 * CNN_D, R], BF16, name=f"kvl_{mid}")
                kv_full = dram.tile([4 * CNN_D, R], BF16, name=f"kvf_{mid}")
                nc.sync.dma_start(
                    kv_loc[0:512, :].rearrange("(c p) j -> p c j", p=P),
                    kTl[:])
                nc.sync.dma_start(
                    kv_loc[512:1024, :].rearrange("(c p) j -> p c j", p=P),
                    vl[:])
                nc.gpsimd.collective_compute(
                    "AllGather", OP.bypass, replica_groups=GROUPS,
                    ins=[kv_loc.opt()], outs=[kv_full.opt()])
                return qT, kv_full

            def mha_attn(x_sb, qT, kv_full, wo, bo, mid):
                kT = spm.tile([P, 4, S], BF16, tag="kT", name=f"kT_{mid}")
                vpk = spm.tile([P, 8, H, 33], BF16, tag="vpk",
                               name=f"vpk_{mid}")
                nc.vector.memset(vpk[:, :, :, 32:33], 1.0)
                for r in range(2):
                    nc.sync.dma_start(
                        kT[:, :, 512 * r: 512 * (r + 1)],
                        kv_full[1024 * r: 1024 * r + 512, :]
                        .rearrange("(c p) j -> p c j", p=P))
                    vpl = spm.tile([P, 4, CNN_D], BF16, tag="vpl", bufs=1,
                                   name=f"vpl_{mid}_{r}")
                    nc.sync.dma_start(
                        vpl[:],
                        kv_full[1024 * r + 512: 1024 * (r + 1), :]
                        .rearrange("(c p) f -> p c f", p=P))
                    for h in range(H):
                        nc.vector.tensor_copy(
                            vpk[:, 4 * r: 4 * r + 4, h, 0:32],
                            vpl[:, :, 32 * h: 32 * h + 32])
                attnT = spm.tile([P, 4, R], BF16, tag="attnT",
                                 name=f"at_{mid}")
                K.attention(qT, kT, vpk, attnT, 32, S, mid)
                xo = front.tile([P, 4, R], BF16, tag="xs", bufs=2,
                                name=f"x_{mid}")
                for m in range(4):
                    ps = K.pps.tile([P, 512], F32, tag="dense", bufs=2,
                                    name=f"pso_{mid}_{m}")
                    for k in range(4):
                        nc.tensor.matmul(
                            ps[:], lhsT=wo[:, k, 128 * m:128 * (m + 1)],
                            rhs=attnT[:, k, :], start=(k == 0), stop=(k == 3))
                    t = spm.tile([P, 512], BF16, tag="otmp", bufs=1,
                                 name=f"ot_{mid}_{m}")
                    nc.vector.tensor_scalar(t[:], ps[:],
                                            scalar1=bo[:, m:m + 1],
                                            scalar2=None, op0=OP.add)
                    nc.vector.tensor_add(xo[:, m, :], t[:], x_sb[:, m, :])
                return xo

            # ---- MHA1 (l2b bridge overlaps the CC1 gather)
            w_saqkv = load_w(wAB, "saqkv", 4, 1536, tag="wqkv", bufs=1)
            w_sao = load_w(wAB, "sao", 4, 512, tag="wsq", bufs=1)
            qA, kvfA = mha_front(x0, w_saqkv, bias["b_saq"], bias["b_sak"],
                                 bias["b_sav"], "m1")
            w_l2b = load_w(wC, "l2b", 6, BR_D, tag="wc", bufs=1)
            bridge(xl, w_l2b, "l2b", 8, BR_D, aug_l, "l2b", kch=6)
            x1 = mha_attn(x0, qA, kvfA, w_sao, bias["b_sao"], "m1")

            # ---- MHA2 front; phys K/V(l) projections overlap its gather
            w_taqkv = load_w(wAB, "taqkv", 4, 1536, tag="wqkv", bufs=1)
            w_tao = load_w(wAB, "tao", 4, 512, tag="wsq", bufs=1)
            qB, kvfB = mha_front(x1, w_taqkv, bias["b_taq"], bias["b_tak"],
                                 bias["b_tav"], "m2")

            w_pk = load_w(wD, "pk", 8, BR_D, tag="wbig", bufs=2)
            w_pv = load_w(wD, "pv", 8, BR_D, tag="wbig", bufs=2)

            def phys_kv(aug, sid):
                kTl = spD.tile([P, 8, R], BF16, tag="pkTl", bufs=1,
                               name=f"pkTl_{sid}")
                vl = spD.tile([P, 4, BR_D], BF16, tag="pvl", bufs=1,
                              name=f"pvl_{sid}")
                K.linear_fm(kTl, w_pk, aug, bias["b_pk"], 8, 8,
                            uid=f"pk{sid}")
                K.vproj_rm(vl, w_pv, aug, bias["b_pv"], 8, 4, BR_D,
                           uid=f"pv{sid}")
                loc = dram.tile([2048, 512], BF16, name=f"pb_{sid}")
                full = dram.tile([4096, 512], BF16, name=f"pf_{sid}")
                nc.sync.dma_start(
                    loc[0:1024, :].rearrange("(c p) j -> p c j", p=P), kTl[:])
                nc.sync.dma_start(
                    loc[1024:2048, :].rearrange("(c p u) j -> p c u j",
                                                p=P, u=2),
                    vl[:].rearrange("p c (u j) -> p c u j", u=2))
                nc.gpsimd.collective_compute(
                    "AllGather", OP.bypass, replica_groups=GROUPS,
                    ins=[loc.opt()], outs=[full.opt()])
                return full

            pf_l = phys_kv(aug_l, "l")

            x2 = mha_attn(x1, qB, kvfB, w_tao, bias["b_tao"], "m2")

            # c2b bridge
            w_c2b = load_w(wC, "c2b", 4, BR_D, tag="wc", bufs=1)
            bridge(x2, w_c2b, "c2b", 8, BR_D, aug_c, "c2b", kch=4)

            pf_c = phys_kv(aug_c, "c")

            # phys-token K/V (local; identical on both pair members)
            for m in range(8):
                ps = K.pps.tile([P, 512], F32, tag="dense", bufs=2,
                                name=f"pspk_{m}")
                for k in range(8):
                    nc.tensor.matmul(ps[:, 0:3],
                                     lhsT=w_pk[:, k, 128 * m:128 * (m + 1)],
                                     rhs=physT[:, k, :],
                                     start=(k == 0), stop=(k == 7))
                nc.vector.tensor_scalar(kT_phys[:, m, :], ps[:, 0:3],
                                        scalar1=bias["b_pk"][:, m:m + 1],
                                        scalar2=None, op0=OP.add)
            for nn in range(2):
                ps = K.pps.tile([P, 512], F32, tag="dense", bufs=2,
                                name=f"pspv_{nn}")
                for k in range(8):
                    nc.tensor.matmul(ps[0:3, :], lhsT=physT[:, k, :],
                                     rhs=w_pv[:, k, 512 * nn: 512 * (nn + 1)],
                                     start=(k == 0), stop=(k == 7))
                nc.vector.tensor_add(
                    v_phys[:, 512 * nn:512 * (nn + 1)], ps[0:3, :],
                    bias["b_pv"][0:3, 512 * nn:512 * (nn + 1)])

            w_pq = load_w(wD, "pq", 8, BR_D, tag="wbig", bufs=2)
            K.linear_fm(q_c, w_pq, aug_c, bias["b_pq"], 8, 8, uid="qc")
            K.linear_fm(q_l, w_pq, aug_l, bias["b_pq"], 8, 8, uid="ql")

    # ======================================================== phys attns
    with tc.tile_pool(name="w_e", bufs=1) as wE, \
         tc.tile_pool(name="sp_e", bufs=1) as spE:
        w_po = load_w(wE, "po", 8, BR_D)

        def phys_gather_in(full, sid):
            kT = spE.tile([P, 8, NK], BF16, tag="physkT", bufs=1,
                          name=f"kTf_{sid}")
            vpl = spE.tile([P, 8, BR_D], BF16, tag="physvpl", bufs=1,
                           name=f"vplf_{sid}")
            for r in range(2):
                nc.sync.dma_start(
                    kT[:, :, 512 * r: 512 * (r + 1)],
                    full[2048 * r: 2048 * r + 1024, :]
                    .rearrange("(c p) j -> p c j", p=P))
                nc.sync.dma_start(
                    vpl[:, 4 * r: 4 * (r + 1), :]
                    .rearrange("p c (u j) -> p c u j", u=2),
                    full[2048 * r + 1024: 2048 * (r + 1), :]
                    .rearrange("(c p u) j -> p c u j", p=P, u=2))
            nc.vector.tensor_copy(kT[:, :, 1024:1027], kT_phys[:])
            vpk = spE.tile([P, 9, H, 65], BF16, tag="physvpk", bufs=1,
                           name=f"vpk_{sid}")
            nc.vector.memset(vpk[:, :, :, 64:65], 1.0)
            for h in range(H):
                nc.vector.tensor_copy(vpk[:, 0:8, h, 0:64],
                                      vpl[:, :, 64 * h: 64 * h + 64])
                nc.vector.tensor_copy(vpk[0:3, 8, h, 0:64],
                                      v_phys[:, 64 * h: 64 * h + 64])
            return kT, vpk

        def phys_attn(qT, kT, vpk, residual, out_res, sid):
            attnT = spE.tile([P, 8, R], BF16, tag="pattnT", bufs=2,
                             name=f"pat_{sid}")
            K.attention(qT, kT, vpk, attnT, 64, NK, f"p{sid}")
            pre = spE.tile([P, 8, R], BF16, tag="pattnT", bufs=2,
                           name=f"pre_{sid}")
            for m in range(8):
                ps = K.pps.tile([P, 512], F32, tag="dense", bufs=2,
                                name=f"pso_{sid}_{m}")
                for k in range(8):
                    nc.tensor.matmul(
                        ps[:], lhsT=w_po[:, k, 128 * m:128 * (m + 1)],
                        rhs=attnT[:, k, :], start=(k == 0), stop=(k == 7))
                t = spE.tile([P, 512], BF16, tag="potmp", bufs=1,
                             name=f"pot_{sid}_{m}")
                nc.vector.tensor_scalar(t[:], ps[:],
                                        scalar1=bias["b_po"][:, m:m + 1],
                                        scalar2=None, op0=OP.add)
                nc.vector.tensor_add(pre[:, m, :], t[:], residual[:, m, :])
            K.layernorm_fm(out_res, pre, bias["g_pa"], bias["e_pa"], 8,
                           BR_D, f"pa_{sid}")

        def final_bridge(x_sb, wname, mch, D, out_d, uid):
            wF = load_w(wE, wname, 8, D, tag="wF", bufs=1)
            gf = spE.tile([P, 8, R], BF16, tag="gF", bufs=1,
                          name=f"gF_{uid}")
            K.linear_fm(gf, wF, x_sb, bias[f"b_{wname}"], 8, mch,
                        func=AF.Gelu, uid=f"fb_{uid}")
            out_r = out_d.rearrange("(c p) j -> p c j", p=P)

            def out_fn(m):
                t = spE.tile([P, 512], F32, tag="oF", bufs=2,
                             name=f"oFc_{uid}_{m}")
                out_fn.pending.append((m, t))
                return t[:]
            out_fn.pending = []
            K.layernorm_fm(None, gf, bias[f"g_{wname}"], bias[f"e_{wname}"],
                           mch, D, f"f_{uid}", out_fn=out_fn)
            for m, t in out_fn.pending:
                nc.sync.dma_start(out_r[:, m, :], t[:])

        kT_l, vpk_l = phys_gather_in(pf_l, "l")
        cnn_att = spE.tile([P, 8, R], BF16, tag="attres", bufs=1,
                           name="cnn_att")
        phys_attn(q_c, kT_l, vpk_l, aug_c, cnn_att, "c")
        final_bridge(cnn_att, "c2l", 6, LLM_D, d["out_llm"], "c2l")

        kT_c, vpk_c = phys_gather_in(pf_c, "c")
        llm_att = spE.tile([P, 8, R], BF16, tag="attres", bufs=1,
                           name="llm_att")
        phys_attn(q_l, kT_c, vpk_c, aug_l, llm_att, "l")
        final_bridge(llm_att, "l2c", 4, CNN_D, d["out_cnn"], "l2c")


def build_kernel():
    nc = bacc.Bacc("TRN2", target_bir_lowering=False, debug=False,
                   num_devices=N_CORES)

    def din(name, shape, dt=BF16):
        return nc.dram_tensor(name, shape, dt, kind="ExternalInput")

    d = {}
    d["xc"] = din("xc", [CNN_D, R])
    d["xl"] = din("xl", [LLM_D, R])
    for n, sh in [("saqkv", [CNN_D, 3 * CNN_D]), ("sao", [CNN_D, CNN_D]),
                  ("taqkv", [CNN_D, 3 * CNN_D]), ("tao", [CNN_D, CNN_D]),
                  ("c2b", [CNN_D, BR_D]), ("l2b", [LLM_D, BR_D]),
                  ("pq", [BR_D, BR_D]), ("pk", [BR_D, BR_D]),
                  ("pv", [BR_D, BR_D]), ("po", [BR_D, BR_D]),
                  ("c2l", [BR_D, LLM_D]), ("l2c", [BR_D, CNN_D])]:
        d[f"w_{n}"] = din(f"w_{n}", sh)
    d["physT"] = din("physT", [BR_D, 3])
    for n, c in [("b_saq", 4), ("b_sak", 4), ("b_sao", 4), ("b_taq", 4),
                 ("b_tak", 4), ("b_tao", 4), ("b_c2b", 8), ("b_l2b", 8),
                 ("b_pq", 8), ("b_pk", 8), ("b_po", 8), ("b_c2l", 6),
                 ("b_l2c", 4), ("g_c2b", 8), ("e_c2b", 8), ("g_l2b", 8),
                 ("e_l2b", 8), ("g_pa", 8), ("e_pa", 8), ("g_c2l", 6),
                 ("e_c2l", 6), ("g_l2c", 4), ("e_l2c", 4),
                 ]:
        d[n] = din(n, [P, c], F32)
    for n, c in [("b_sav", CNN_D), ("b_tav", CNN_D), ("b_pv", BR_D)]:
        d[n] = din(n, [P, c], BF16)
    d["out_cnn"] = nc.dram_tensor("out_cnn", [CNN_D, R], F32,
                                  kind="ExternalOutput")
    d["out_llm"] = nc.dram_tensor("out_llm", [LLM_D, R], F32,
                                  kind="ExternalOutput")

    with tile.TileContext(nc) as tc:
        with ExitStack() as ctx:
            K = Ker(nc, tc, d)
            _build_body(K, ctx)
    nc.compile()
    return nc


# ------------------------------------------------------------------- host

_NC_CACHE = None


def _get_nc():
    global _NC_CACHE
    if _NC_CACHE is None:
        _NC_CACHE = build_kernel()
    return _NC_CACHE


def _pack_host(inputs):
    f32 = np.float32

    def wT(x):
        return np.ascontiguousarray(np.asarray(x, f32).T).astype(bf16)

    def pp(x, cols):
        return np.ascontiguousarray(np.asarray(x, f32).reshape(cols, P).T)

    def rep(x):
        x = np.asarray(x, f32)
        return np.ascontiguousarray(np.broadcast_to(x[None, :], (P, len(x))))

    i = {k: np.asarray(v) for k, v in inputs.items()}
    qs, qt = i["sa_bqkv"], i["ta_bqkv"]
    phys = np.stack([i["e_emb"][0, 0], i["m_emb"][0, 0], i["p_emb"][0, 0]])

    shared = dict(
        w_saqkv=wT(i["sa_wqkv"]), w_sao=wT(i["sa_wo"]),
        w_taqkv=wT(i["ta_wqkv"]), w_tao=wT(i["ta_wo"]),
        w_c2b=wT(i["c2b_w"]), w_l2b=wT(i["l2b_w"]),
        w_pq=wT(i["pq_w"]), w_pk=wT(i["pk_w"]), w_pv=wT(i["pv_w"]),
        w_po=wT(i["po_w"]), w_c2l=wT(i["c2l_w"]), w_l2c=wT(i["l2c_w"]),
        physT=np.ascontiguousarray(phys.astype(f32).T).astype(bf16),
        b_saq=pp(qs[0:512], 4), b_sak=pp(qs[512:1024], 4),
        b_sav=rep(qs[1024:1536]).astype(bf16), b_sao=pp(i["sa_bo"], 4),
        b_taq=pp(qt[0:512], 4), b_tak=pp(qt[512:1024], 4),
        b_tav=rep(qt[1024:1536]).astype(bf16), b_tao=pp(i["ta_bo"], 4),
        b_c2b=pp(i["c2b_b"], 8), b_l2b=pp(i["l2b_b"], 8),
        b_pq=pp(i["pq_b"], 8), b_pk=pp(i["pk_b"], 8),
        b_pv=rep(i["pv_b"]).astype(bf16), b_po=pp(i["po_b"], 8),
        b_c2l=pp(i["c2l_b"], 6), b_l2c=pp(i["l2c_b"], 4),
        g_c2b=pp(i["c2b_g"], 8), e_c2b=pp(i["c2b_be"], 8),
        g_l2b=pp(i["l2b_g"], 8), e_l2b=pp(i["l2b_be"], 8),
        g_pa=pp(i["pa_g"], 8), e_pa=pp(i["pa_be"], 8),
        g_c2l=pp(i["c2l_g"], 6), e_c2l=pp(i["c2l_be"], 6),
        g_l2c=pp(i["l2c_g"], 4), e_l2c=pp(i["l2c_be"], 4),
    )
    in_maps = []
    for c in range(N_CORES):
        b, hh = c // 2, c % 2
        m = dict(shared)
        m["xc"] = wT(i["enhanced_cnn_features"][b, hh * R:(hh + 1) * R])
        m["xl"] = wT(i["llm_features"][b, hh * R:(hh + 1) * R])
        in_maps.append(m)
    return in_maps


def kernel(**inputs):
    nc = _get_nc()
    in_maps = _pack_host(inputs)
    res = run_bass_kernel_spmd(nc, in_maps, list(range(N_CORES)))
    cnn = np.empty((B, S, CNN_D), np.float32)
    llm = np.empty((B, S, LLM_D), np.float32)
    for c in range(N_CORES):
        b, hh = c // 2, c % 2
        cnn[b, hh * R:(hh + 1) * R] = res.results[c]["out_cnn"].T
        llm[b, hh * R:(hh + 1) * R] = res.results[c]["out_llm"].T
    return (cnn, llm)


# revision 28
# speedup vs baseline: 1.3572x; 1.3572x over previous
"""Trainium2 Bass kernel for nn_DatacubeLLMBridge (dense_transformer).

Sharding: 8 cores = 4 batches x 2 sequence-halves. Core c owns batch c//2,
rows [(c%2)*512, (c%2+1)*512) of S=1024. Weights replicated. Cross-core
traffic: AllGather of K/V within pairs [[0,1],[2,3],[4,5],[6,7]] (4 total).

Layout: activations are FEATURE-major in SBUF: x^T = [feat chunks of 128
partitions, rows in free dim]. Matmuls contract the partition dim. Weights
host-pretransposed to W^T=[fin,fout] bf16. Row-major V (for attention AV)
is produced by swapping matmul operands (lhsT = x^T row-chunk).

Softmax: the reference's energy/mass biases are constant per query row and
softmax is shift-invariant per row, so they cancel exactly. Score
magnitudes are O(1) here (0.02-scale weights), so exp without
max-subtraction is safe. Softmax denominators come from a ones-augmented V
column; the normalization is folded into the AV-PSUM evict.

PSUM budget (8 banks): dense [128,512]x2 + scores [128,4,512]x1 + avt
[65,512]x2 = 8.
"""

from contextlib import ExitStack

import numpy as np
import ml_dtypes

import concourse.bass as bass
import concourse.mybir as mybir
import concourse.tile as tile
from concourse import bacc
from concourse.bass_utils import run_bass_kernel_spmd

F32 = mybir.dt.float32
BF16 = mybir.dt.bfloat16
AF = mybir.ActivationFunctionType
OP = mybir.AluOpType


# Unify Exp/Ln into one ACT table set (natural_log_exp_and_others) so the
# softmax-reciprocal ln/exp does not thrash ACT_TABLE_LOADs. Contents-only
# edit; set order (act_func_set_id indices) is preserved.
import functools as _ft
import concourse.hw_specs as _hw
import concourse.bacc as _bacc_mod

_ORIG_GAT = _hw.get_activation_tables


@_ft.cache
def _patched_gat(arch):
    t = dict(_ORIG_GAT(arch))
    drop = {AF.Exp, AF.Ln}
    for name in ("exp_and_others", "natural_log"):
        if name in t:
            t[name] = set(t[name]) - drop
    return t


_hw.get_activation_tables = _patched_gat
_bacc_mod.get_activation_tables = _patched_gat

P = 128
B, S, CNN_D, LLM_D, BR_D, H = 4, 1024, 512, 768, 1024, 16
R = 512            # rows per core
N_CORES = 8
GROUPS = [[0, 1], [2, 3], [4, 5], [6, 7]]
EPS = 1e-5
NK = S + 3         # phys attention keys

bf16 = ml_dtypes.bfloat16


class Ker:
    """Holds build state so helpers can share pools/constants."""

    def __init__(self, nc, tc, d):
        self.nc, self.tc, self.d = nc, tc, d
        self.rr_recip = True

    # ------------------------------------------------------------ helpers
    def linear_fm(self, out_sb, wT, x_sb, bias, kch, mch, func=AF.Copy,
                  wcol0=0, uid=""):
        """out^T[:,m,:] = func(sum_k wT[:,k,col].T @ x[:,k,:] + bias[:,m])."""
        nc = self.nc
        ncols = x_sb.shape[-1]
        for m in range(mch):
            ps = self.pps.tile([P, 512], F32, tag="dense", bufs=2,
                               name=f"psd_{uid}_{m}")
            for k in range(kch):
                nc.tensor.matmul(
                    ps[:, :ncols],
                    lhsT=wT[:, k, wcol0 + 128 * m: wcol0 + 128 * (m + 1)],
                    rhs=x_sb[:, k, :],
                    start=(k == 0), stop=(k == kch - 1))
            if func == AF.Copy:
                nc.vector.tensor_scalar(out_sb[:, m, :], ps[:, :ncols],
                                        scalar1=bias[:, m:m + 1],
                                        scalar2=None, op0=OP.add)
            else:
                nc.scalar.activation(out_sb[:, m, :], ps[:, :ncols], func,
                                     bias=bias[:, m:m + 1])

    def vproj_rm(self, out_sb, wT, x_sb, bias_rep, kch, rch, fout,
                 wcol0=0, uid=""):
        """Row-major projection: out[:,rc,:] = x-rows @ W^T + bias."""
        nc = self.nc
        for rc in range(rch):
            for nn in range((fout + 511) // 512):
                n0, n1 = nn * 512, min(fout, (nn + 1) * 512)
                ps = self.pps.tile([P, 512], F32, tag="dense", bufs=2,
                                   name=f"psv_{uid}_{rc}_{nn}")
                for k in range(kch):
                    nc.tensor.matmul(
                        ps[:, : n1 - n0],
                        lhsT=x_sb[:, k, 128 * rc: 128 * (rc + 1)],
                        rhs=wT[:, k, wcol0 + n0: wcol0 + n1],
                        start=(k == 0), stop=(k == kch - 1))
                nc.vector.tensor_add(out_sb[:, rc, n0:n1], ps[:, : n1 - n0],
                                     bias_rep[:, n0:n1])

    def layernorm_fm(self, out_sb, in_sb, gamma, beta, mch, D, uid,
                     out_fn=None):
        """LN over the feature (partition-chunk) dim; in_sb [128,mch,512].
        If out_fn is given it must return the destination AP for chunk m."""
        nc, sp = self.nc, self.spln
        sq = sp.tile([P, 8, 512], BF16, tag="ln_sq", name=f"lnsq_{uid}")
        nc.vector.tensor_mul(sq[:, :mch, :], in_sb[:, :mch, :],
                             in_sb[:, :mch, :])
        ps_s = self.pps.tile([1, 512], F32, tag="dense", bufs=2,
                             name=f"lns_{uid}")
        ps_q = self.pps.tile([1, 512], F32, tag="dense", bufs=2,
                             name=f"lnq_{uid}")
        for k in range(mch):
            nc.tensor.matmul(ps_s[:], lhsT=self.ones_bf[:, 0:1],
                             rhs=in_sb[:, k, :],
                             start=(k == 0), stop=(k == mch - 1))
        for k in range(mch):
            nc.tensor.matmul(ps_q[:], lhsT=self.ones_bf[:, 0:1],
                             rhs=sq[:, k, :],
                             start=(k == 0), stop=(k == mch - 1))
        sm = sp.tile([1, 2, 512], F32, tag="ln_sm", name=f"lnsm_{uid}")
        mean, msq = sm[0:1, 0, :], sm[0:1, 1, :]
        rstd = sp.tile([1, 512], F32, tag="ln_rstd", name=f"lnrs_{uid}")
        mr = sp.tile([1, 512], F32, tag="ln_mr", name=f"lnmr_{uid}")
        nc.vector.tensor_scalar_mul(mean, ps_s[:], 1.0 / D)
        nc.vector.tensor_scalar_mul(msq, ps_q[:], 1.0 / D)
        nc.vector.tensor_mul(rstd[:], mean, mean)      # rstd = mean^2 (tmp)
        nc.vector.tensor_sub(msq, msq, rstd[:])        # msq  = var
        nc.scalar.activation(msq, msq, AF.Sqrt, bias=self.eps_t[0:1, :])
        nc.vector.reciprocal(rstd[:], msq)
        nc.vector.tensor_mul(mr[:], mean, rstd[:])
        bc = sp.tile([P, 2, 512], BF16, tag="ln_bc", name=f"lnbc_{uid}")
        for j, src in ((0, rstd[:]), (1, mr[:])):
            ps_b = self.pps.tile([P, 512], F32, tag="dense", bufs=2,
                                 name=f"lnb_{uid}_{j}")
            nc.tensor.matmul(ps_b[:], lhsT=self.ones_1x128[:], rhs=src,
                             start=True, stop=True)
            nc.vector.tensor_copy(bc[:, j, :], ps_b[:])
        for m in range(mch):
            t = sp.tile([P, 512], BF16, tag="ln_t", bufs=1,
                        name=f"lnt_{uid}_{m}")
            nc.vector.tensor_mul(t[:], in_sb[:, m, :], bc[:, 0, :])
            nc.vector.tensor_sub(t[:], t[:], bc[:, 1, :])
            dst = out_fn(m) if out_fn is not None else out_sb[:, m, :]
            nc.vector.tensor_scalar(dst, t[:],
                                    scalar1=gamma[:, m:m + 1],
                                    scalar2=beta[:, m:m + 1],
                                    op0=OP.mult, op1=OP.add)

    def attention(self, exp_pool, exp_bufs, qT, kT_full, v_packed, attnoutT,
                  dh, n_keys, uid):
        """Phase-separated: all scores+exp for a (group, pair), then AV,
        then the normalize chain. Softmax reciprocal alternates DVE <->
        ACT(ln/exp); ACT table sets unified via get_activation_tables
        patch."""
        nc, sp = self.nc, self.spat
        E = qT.shape[1] * P
        hpc = P // dh
        kc_full, rag = n_keys // P, n_keys % P
        kcN = kc_full + (1 if rag else 0)
        scale = 1.0 / float(np.sqrt(dh))
        npair = hpc // 2
        for g in range(E // P):
            exps = [exp_pool.tile([P, 9, 2, 512], BF16, tag="expS",
                                  bufs=exp_bufs,
                                  name=f"exp_{uid}_{g}_{a}")
                    for a in range(npair)]
            for kc in range(kcN):
                kk = P if kc < kc_full else rag
                for a in range(npair):
                    ps_s = self.pps.tile([P, 2, 512], F32, tag="sc", bufs=2,
                                         name=f"sc_{uid}_{g}_{kc}_{a}")
                    for i in range(2):
                        ho = (2 * a + i) * dh
                        nc.tensor.matmul(
                            ps_s[:kk, i, :],
                            lhsT=kT_full[ho:ho + dh, g, kc * P: kc * P + kk],
                            rhs=qT[ho:ho + dh, g, :],
                            start=True, stop=True,
                            tile_position=(ho, 0))
                    nc.scalar.activation(
                        exps[a][:kk, kc, :, :], ps_s[:kk, :, :],
                        AF.Exp, scale=scale)
            for a in range(npair):
                ps_avs = []
                rps = []
                for i in range(2):
                    h = g * hpc + 2 * a + i
                    ps_av = self.pps.tile([65, 512], F32, tag="avt", bufs=2,
                                          name=f"av_{uid}_{g}_{a}_{i}")
                    for kc in range(kcN):
                        kk = P if kc < kc_full else rag
                        nc.tensor.matmul(
                            ps_av[: dh + 1, :],
                            lhsT=v_packed[:kk, kc, h, :],
                            rhs=exps[a][:kk, kc, i, :],
                            start=(kc == 0), stop=(kc == kcN - 1))
                    rp = sp.tile([1, 512], F32, tag="recip", bufs=2,
                                 name=f"rp_{uid}_{g}_{a}_{i}")
                    if self.rr_recip:
                        nc.vector.reciprocal(rp[:], ps_av[dh:dh + 1, :])
                    else:
                        nc.scalar.activation(rp[:], ps_av[dh:dh + 1, :],
                                             AF.Ln)
                        nc.scalar.activation(rp[:], rp[:], AF.Exp,
                                             scale=-1.0)
                    self.rr_recip = not self.rr_recip
                    ps_avs.append(ps_av)
                    rps.append(rp)
                ps_bc = self.pps.tile([P, 512], F32, tag="dense", bufs=2,
                                      name=f"bc_{uid}_{g}_{a}")
                for i in range(2):
                    ho = (2 * a + i) * dh
                    nc.tensor.matmul(ps_bc[ho:ho + dh, :],
                                     lhsT=self.ones_1x128[0:1, 0:dh],
                                     rhs=rps[i][:],
                                     start=True, stop=True,
                                     tile_position=(0, ho))
                bo0 = 2 * a * dh
                bc_sb = sp.tile([P, 512], BF16, tag="bc_sb", bufs=2,
                                name=f"bcs_{uid}_{g}_{a}")
                nc.scalar.activation(bc_sb[bo0:bo0 + 2 * dh, :],
                                     ps_bc[bo0:bo0 + 2 * dh, :], AF.Copy)
                for i in range(2):
                    ho = (2 * a + i) * dh
                    nc.vector.tensor_tensor(
                        attnoutT[ho:ho + dh, g, :], ps_avs[i][0:dh, :],
                        bc_sb[ho:ho + dh, :], op=OP.mult)


def _build_body(K, ctx):
    nc, tc, d = K.nc, K.tc, K.d

    # ---------------- persistent pools
    pc = ctx.enter_context(tc.tile_pool(name="consts", bufs=1))
    st = ctx.enter_context(tc.tile_pool(name="stream", bufs=1))
    dram = ctx.enter_context(tc.tile_pool(name="drampool", bufs=1,
                                          space="DRAM"))
    K.pps = ctx.enter_context(tc.tile_pool(name="pspool", bufs=1,
                                           space="PSUM"))
    K.spln = ctx.enter_context(tc.tile_pool(name="sp_ln", bufs=1))
    K.spat = ctx.enter_context(tc.tile_pool(name="sp_att", bufs=1))
    spq = ctx.enter_context(tc.tile_pool(name="sp_q", bufs=1))

    def load_pp(name, cols, dt=F32):
        t = pc.tile([P, cols], dt, name=f"c_{name}")
        nc.sync.dma_start(t[:], d[name][:])
        return t

    bias = {n: load_pp(n, c) for n, c in [
        ("b_saq", 4), ("b_sak", 4), ("b_sao", 4), ("b_taq", 4), ("b_tak", 4),
        ("b_tao", 4), ("b_c2b", 8), ("b_l2b", 8), ("b_pq", 8), ("b_pk", 8),
        ("b_po", 8), ("b_c2l", 6), ("b_l2c", 4), ("g_c2b", 8), ("e_c2b", 8),
        ("g_l2b", 8), ("e_l2b", 8), ("g_pa", 8), ("e_pa", 8), ("g_c2l", 6),
        ("e_c2l", 6), ("g_l2c", 4), ("e_l2c", 4)]}
    for n, c in [("b_sav", CNN_D), ("b_tav", CNN_D), ("b_pv", BR_D)]:
        bias[n] = load_pp(n, c, dt=BF16)

    K.ones_bf = pc.tile([P, 1], BF16, name="ones_bf")
    nc.vector.memset(K.ones_bf[:], 1.0)
    K.ones_1x128 = pc.tile([1, P], F32, name="ones_1x128")
    nc.vector.memset(K.ones_1x128[:], 1.0)
    K.ones_1x128_bf = pc.tile([1, P], BF16, name="ones_1x128_bf")
    nc.vector.memset(K.ones_1x128_bf[:], 1.0)
    K.eps_t = pc.tile([1, 1], F32, name="eps_t")
    nc.vector.memset(K.eps_t[:], EPS)
    physT = pc.tile([P, 8, 3], BF16, name="physT_sb")
    nc.sync.dma_start(physT[:], d["physT"].rearrange("(c p) t -> p c t", p=P))

    def load_w(pool, name, kch, fout, tag=None, bufs=1):
        t = pool.tile([P, kch, fout], BF16, tag=tag or f"w{name}", bufs=bufs,
                      name=f"w_{name}")
        nc.sync.dma_start(t[:],
                          d[f"w_{name}"].rearrange("(c p) n -> p c n", p=P))
        return t

    aug_l = st.tile([P, 8, R], BF16, tag="aug", bufs=2, name="aug_l")
    aug_c = st.tile([P, 8, R], BF16, tag="aug", bufs=2, name="aug_c")
    q_c = spq.tile([P, 8, R], BF16, tag="physq", bufs=2, name="q_c")
    q_l = spq.tile([P, 8, R], BF16, tag="physq", bufs=2, name="q_l")
    kT_phys = spq.tile([P, 8, 3], BF16, name="kT_phys")
    v_phys = spq.tile([3, BR_D], BF16, name="v_phys")

    # ======================================================= front stages
    with tc.tile_pool(name="front", bufs=1) as front, \
         tc.tile_pool(name="w_c", bufs=1) as wC, \
         tc.tile_pool(name="sp_br", bufs=1) as spbr:

        x0 = front.tile([P, 4, R], BF16, tag="xs", bufs=2, name="x0")
        nc.sync.dma_start(x0[:], d["xc"].rearrange("(c p) j -> p c j", p=P))
        xl = front.tile([P, 6, R], BF16, name="xl")
        nc.sync.dma_start(xl[:], d["xl"].rearrange("(c p) j -> p c j", p=P))

        def bridge(x_sb, wt, bname, mch, D, out_sb, uid, kch):
            g = spbr.tile([P, 8, R], BF16, tag="gelu", bufs=1,
                          name=f"gelu_{uid}")
            K.linear_fm(g, wt, x_sb, bias[f"b_{bname}"], kch, mch,
                        func=AF.Gelu, uid=f"br_{uid}")
            K.layernorm_fm(out_sb, g, bias[f"g_{bname}"], bias[f"e_{bname}"],
                           mch, D, uid)

        with tc.tile_pool(name="w_ab", bufs=1) as wAB, \
             tc.tile_pool(name="sp_mha", bufs=1) as spm:

            def mha_front(x_sb, wqkv, bq, bk, bv_rep, mid):
                qT = spm.tile([P, 4, R], BF16, tag="qT", name=f"qT_{mid}")
                kTl = spm.tile([P, 4, R], BF16, tag="kTl", name=f"kTl_{mid}")
                vl = spm.tile([P, 4, CNN_D], BF16, tag="vl", name=f"vl_{mid}")
                K.linear_fm(qT, wqkv, x_sb, bq, 4, 4, wcol0=0, uid=f"q{mid}")
                K.linear_fm(kTl, wqkv, x_sb, bk, 4, 4, wcol0=512,
                            uid=f"k{mid}")
                K.vproj_rm(vl, wqkv, x_sb, bv_rep, 4, 4, CNN_D, wcol0=1024,
                           uid=f"v{mid}")
                kv_loc = dram.tile([2 * CNN_D, R], BF16, name=f"kvl_{mid}")
                kv_full = dram.tile([4 * CNN_D, R], BF16, name=f"kvf_{mid}")
                nc.sync.dma_start(
                    kv_loc[0:512, :].rearrange("(c p) j -> p c j", p=P),
                    kTl[:])
                nc.sync.dma_start(
                    kv_loc[512:1024, :].rearrange("(c p) j -> p c j", p=P),
                    vl[:])
                nc.gpsimd.collective_compute(
                    "AllGather", OP.bypass, replica_groups=GROUPS,
                    ins=[kv_loc.opt()], outs=[kv_full.opt()])
                return qT, kv_full

            def mha_attn(x_sb, qT, kv_full, wo, bo, mid):
                kT = spm.tile([P, 4, S], BF16, tag="kT", name=f"kT_{mid}")
                vpk = spm.tile([P, 8, H, 33], BF16, tag="vpk",
                               name=f"vpk_{mid}")
                nc.vector.memset(vpk[:, :, :, 32:33], 1.0)
                for r in range(2):
                    nc.sync.dma_start(
                        kT[:, :, 512 * r: 512 * (r + 1)],
                        kv_full[1024 * r: 1024 * r + 512, :]
                        .rearrange("(c p) j -> p c j", p=P))
                    vpl = spm.tile([P, 4, CNN_D], BF16, tag="vpl", bufs=1,
                                   name=f"vpl_{mid}_{r}")
                    nc.sync.dma_start(
                        vpl[:],
                        kv_full[1024 * r + 512: 1024 * (r + 1), :]
                        .rearrange("(c p) f -> p c f", p=P))
                    for h in range(H):
                        nc.vector.tensor_copy(
                            vpk[:, 4 * r: 4 * r + 4, h, 0:32],
                            vpl[:, :, 32 * h: 32 * h + 32])
                attnT = spm.tile([P, 4, R], BF16, tag="attnT",
                                 name=f"at_{mid}")
                K.attention(spm, 2, qT, kT, vpk, attnT, 32, S, mid)
                xo = front.tile([P, 4, R], BF16, tag="xs", bufs=2,
                                name=f"x_{mid}")
                for m in range(4):
                    ps = K.pps.tile([P, 512], F32, tag="dense", bufs=2,
                                    name=f"pso_{mid}_{m}")
                    for k in range(4):
                        nc.tensor.matmul(
                            ps[:], lhsT=wo[:, k, 128 * m:128 * (m + 1)],
                            rhs=attnT[:, k, :], start=(k == 0), stop=(k == 3))
                    t = spm.tile([P, 512], BF16, tag="otmp", bufs=1,
                                 name=f"ot_{mid}_{m}")
                    nc.vector.tensor_scalar(t[:], ps[:],
                                            scalar1=bo[:, m:m + 1],
                                            scalar2=None, op0=OP.add)
                    nc.vector.tensor_add(xo[:, m, :], t[:], x_sb[:, m, :])
                return xo

            # ---- MHA1 (l2b bridge overlaps the CC1 gather)
            w_saqkv = load_w(wAB, "saqkv", 4, 1536, tag="wqkv", bufs=1)
            w_sao = load_w(wAB, "sao", 4, 512, tag="wsq", bufs=1)
            qA, kvfA = mha_front(x0, w_saqkv, bias["b_saq"], bias["b_sak"],
                                 bias["b_sav"], "m1")
            w_l2b = load_w(wC, "l2b", 6, BR_D, tag="wc", bufs=1)
            bridge(xl, w_l2b, "l2b", 8, BR_D, aug_l, "l2b", kch=6)
            x1 = mha_attn(x0, qA, kvfA, w_sao, bias["b_sao"], "m1")

            # ---- MHA2
            w_taqkv = load_w(wAB, "taqkv", 4, 1536, tag="wqkv", bufs=1)
            w_tao = load_w(wAB, "tao", 4, 512, tag="wsq", bufs=1)
            qB, kvfB = mha_front(x1, w_taqkv, bias["b_taq"], bias["b_tak"],
                                 bias["b_tav"], "m2")
            x2 = mha_attn(x1, qB, kvfB, w_tao, bias["b_tao"], "m2")

        # ================================== phys projections + c2b bridge
        with tc.tile_pool(name="w_d", bufs=1) as wD, \
             tc.tile_pool(name="sp_d", bufs=1) as spD:
            w_pk = load_w(wD, "pk", 8, BR_D, tag="wbig", bufs=2)
            w_pv = load_w(wD, "pv", 8, BR_D, tag="wbig", bufs=2)

            def phys_kv(aug, sid):
                kTl = spD.tile([P, 8, R], BF16, tag="pkTl", bufs=1,
                               name=f"pkTl_{sid}")
                vl = spD.tile([P, 4, BR_D], BF16, tag="pvl", bufs=1,
                              name=f"pvl_{sid}")
                K.linear_fm(kTl, w_pk, aug, bias["b_pk"], 8, 8,
                            uid=f"pk{sid}")
                K.vproj_rm(vl, w_pv, aug, bias["b_pv"], 8, 4, BR_D,
                           uid=f"pv{sid}")
                loc = dram.tile([2048, 512], BF16, name=f"pb_{sid}")
                full = dram.tile([4096, 512], BF16, name=f"pf_{sid}")
                nc.sync.dma_start(
                    loc[0:1024, :].rearrange("(c p) j -> p c j", p=P), kTl[:])
                nc.sync.dma_start(
                    loc[1024:2048, :].rearrange("(c p u) j -> p c u j",
                                                p=P, u=2),
                    vl[:].rearrange("p c (u j) -> p c u j", u=2))
                nc.gpsimd.collective_compute(
                    "AllGather", OP.bypass, replica_groups=GROUPS,
                    ins=[loc.opt()], outs=[full.opt()])
                return full

            pf_l = phys_kv(aug_l, "l")

            # c2b bridge
            w_c2b = load_w(wC, "c2b", 4, BR_D, tag="wc", bufs=1)
            bridge(x2, w_c2b, "c2b", 8, BR_D, aug_c, "c2b", kch=4)

            pf_c = phys_kv(aug_c, "c")

            # phys-token K/V (local; identical on both pair members)
            for m in range(8):
                ps = K.pps.tile([P, 512], F32, tag="dense", bufs=2,
                                name=f"pspk_{m}")
                for k in range(8):
                    nc.tensor.matmul(ps[:, 0:3],
                                     lhsT=w_pk[:, k, 128 * m:128 * (m + 1)],
                                     rhs=physT[:, k, :],
                                     start=(k == 0), stop=(k == 7))
                nc.vector.tensor_scalar(kT_phys[:, m, :], ps[:, 0:3],
                                        scalar1=bias["b_pk"][:, m:m + 1],
                                        scalar2=None, op0=OP.add)
            for nn in range(2):
                ps = K.pps.tile([P, 512], F32, tag="dense", bufs=2,
                                name=f"pspv_{nn}")
                for k in range(8):
                    nc.tensor.matmul(ps[0:3, :], lhsT=physT[:, k, :],
                                     rhs=w_pv[:, k, 512 * nn: 512 * (nn + 1)],
                                     start=(k == 0), stop=(k == 7))
                nc.vector.tensor_add(
                    v_phys[:, 512 * nn:512 * (nn + 1)], ps[0:3, :],
                    bias["b_pv"][0:3, 512 * nn:512 * (nn + 1)])

            w_pq = load_w(wD, "pq", 8, BR_D, tag="wbig", bufs=2)
            K.linear_fm(q_c, w_pq, aug_c, bias["b_pq"], 8, 8, uid="qc")
            K.linear_fm(q_l, w_pq, aug_l, bias["b_pq"], 8, 8, uid="ql")

    # ======================================================== phys attns
    with tc.tile_pool(name="w_e", bufs=1) as wE, \
         tc.tile_pool(name="sp_e", bufs=1) as spE:
        w_po = load_w(wE, "po", 8, BR_D)

        def phys_gather_in(full, sid):
            kT = spE.tile([P, 8, NK], BF16, tag="physkT", bufs=1,
                          name=f"kTf_{sid}")
            vpl = spE.tile([P, 8, BR_D], BF16, tag="physvpl", bufs=1,
                           name=f"vplf_{sid}")
            for r in range(2):
                nc.sync.dma_start(
                    kT[:, :, 512 * r: 512 * (r + 1)],
                    full[2048 * r: 2048 * r + 1024, :]
                    .rearrange("(c p) j -> p c j", p=P))
                nc.sync.dma_start(
                    vpl[:, 4 * r: 4 * (r + 1), :]
                    .rearrange("p c (u j) -> p c u j", u=2),
                    full[2048 * r + 1024: 2048 * (r + 1), :]
                    .rearrange("(c p u) j -> p c u j", p=P, u=2))
            nc.vector.tensor_copy(kT[:, :, 1024:1027], kT_phys[:])
            vpk = spE.tile([P, 9, H, 65], BF16, tag="physvpk", bufs=1,
                           name=f"vpk_{sid}")
            nc.vector.memset(vpk[:, :, :, 64:65], 1.0)
            for h in range(H):
                nc.vector.tensor_copy(vpk[:, 0:8, h, 0:64],
                                      vpl[:, :, 64 * h: 64 * h + 64])
                nc.vector.tensor_copy(vpk[0:3, 8, h, 0:64],
                                      v_phys[:, 64 * h: 64 * h + 64])
            return kT, vpk

        def phys_attn(qT, kT, vpk, residual, out_res, sid):
            attnT = spE.tile([P, 8, R], BF16, tag="pattnT", bufs=2,
                             name=f"pat_{sid}")
            K.attention(spE, 1, qT, kT, vpk, attnT, 64, NK, f"p{sid}")
            pre = spE.tile([P, 8, R], BF16, tag="pattnT", bufs=2,
                           name=f"pre_{sid}")
            for m in range(8):
                ps = K.pps.tile([P, 512], F32, tag="dense", bufs=2,
                                name=f"pso_{sid}_{m}")
                for k in range(8):
                    nc.tensor.matmul(
                        ps[:], lhsT=w_po[:, k, 128 * m:128 * (m + 1)],
                        rhs=attnT[:, k, :], start=(k == 0), stop=(k == 7))
                t = spE.tile([P, 512], BF16, tag="potmp", bufs=1,
                             name=f"pot_{sid}_{m}")
                nc.vector.tensor_scalar(t[:], ps[:],
                                        scalar1=bias["b_po"][:, m:m + 1],
                                        scalar2=None, op0=OP.add)
                nc.vector.tensor_add(pre[:, m, :], t[:], residual[:, m, :])
            K.layernorm_fm(out_res, pre, bias["g_pa"], bias["e_pa"], 8,
                           BR_D, f"pa_{sid}")

        def final_bridge(x_sb, wname, mch, D, out_d, uid):
            wF = load_w(wE, wname, 8, D, tag="wF", bufs=1)
            gf = spE.tile([P, 8, R], BF16, tag="gF", bufs=1,
                          name=f"gF_{uid}")
            K.linear_fm(gf, wF, x_sb, bias[f"b_{wname}"], 8, mch,
                        func=AF.Gelu, uid=f"fb_{uid}")
            out_r = out_d.rearrange("(c p) j -> p c j", p=P)

            def out_fn(m):
                t = spE.tile([P, 512], F32, tag="oF", bufs=2,
                             name=f"oFc_{uid}_{m}")
                out_fn.pending.append((m, t))
                return t[:]
            out_fn.pending = []
            K.layernorm_fm(None, gf, bias[f"g_{wname}"], bias[f"e_{wname}"],
                           mch, D, f"f_{uid}", out_fn=out_fn)
            for m, t in out_fn.pending:
                nc.sync.dma_start(out_r[:, m, :], t[:])

        kT_l, vpk_l = phys_gather_in(pf_l, "l")
        cnn_att = spE.tile([P, 8, R], BF16, tag="attres", bufs=1,
                           name="cnn_att")
        phys_attn(q_c, kT_l, vpk_l, aug_c, cnn_att, "c")
        final_bridge(cnn_att, "c2l", 6, LLM_D, d["out_llm"], "c2l")

        kT_c, vpk_c = phys_gather_in(pf_c, "c")
        llm_att = spE.tile([P, 8, R], BF16, tag="attres", bufs=1,
                           name="llm_att")
        phys_attn(q_l, kT_c, vpk_c, aug_l, llm_att, "l")
        final_bridge(llm_att, "l2c", 4, CNN_D, d["out_cnn"], "l2c")


def build_kernel():
    nc = bacc.Bacc("TRN2", target_bir_lowering=False, debug=False,
                   num_devices=N_CORES)

    def din(name, shape, dt=BF16):
        return nc.dram_tensor(name, shape, dt, kind="ExternalInput")

    d = {}
    d["xc"] = din("xc", [CNN_D, R])
    d["xl"] = din("xl", [LLM_D, R])
    for n, sh in [("saqkv", [CNN_D, 3 * CNN_D]), ("sao", [CNN_D, CNN_D]),
                  ("taqkv", [CNN_D, 3 * CNN_D]), ("tao", [CNN_D, CNN_D]),
                  ("c2b", [CNN_D, BR_D]), ("l2b", [LLM_D, BR_D]),
                  ("pq", [BR_D, BR_D]), ("pk", [BR_D, BR_D]),
                  ("pv", [BR_D, BR_D]), ("po", [BR_D, BR_D]),
                  ("c2l", [BR_D, LLM_D]), ("l2c", [BR_D, CNN_D])]:
        d[f"w_{n}"] = din(f"w_{n}", sh)
    d["physT"] = din("physT", [BR_D, 3])
    for n, c in [("b_saq", 4), ("b_sak", 4), ("b_sao", 4), ("b_taq", 4),
                 ("b_tak", 4), ("b_tao", 4), ("b_c2b", 8), ("b_l2b", 8),
                 ("b_pq", 8), ("b_pk", 8), ("b_po", 8), ("b_c2l", 6),
                 ("b_l2c", 4), ("g_c2b", 8), ("e_c2b", 8), ("g_l2b", 8),
                 ("e_l2b", 8), ("g_pa", 8), ("e_pa", 8), ("g_c2l", 6),
                 ("e_c2l", 6), ("g_l2c", 4), ("e_l2c", 4),
                 ]:
        d[n] = din(n, [P, c], F32)
    for n, c in [("b_sav", CNN_D), ("b_tav", CNN_D), ("b_pv", BR_D)]:
        d[n] = din(n, [P, c], BF16)
    d["out_cnn"] = nc.dram_tensor("out_cnn", [CNN_D, R], F32,
                                  kind="ExternalOutput")
    d["out_llm"] = nc.dram_tensor("out_llm", [LLM_D, R], F32,
                                  kind="ExternalOutput")

    with tile.TileContext(nc) as tc:
        with ExitStack() as ctx:
            K = Ker(nc, tc, d)
            _build_body(K, ctx)
    nc.compile()
    return nc


# ------------------------------------------------------------------- host

_NC_CACHE = None


def _get_nc():
    global _NC_CACHE
    if _NC_CACHE is None:
        _NC_CACHE = build_kernel()
    return _NC_CACHE


def _pack_host(inputs):
    f32 = np.float32

    def wT(x):
        return np.ascontiguousarray(np.asarray(x, f32).T).astype(bf16)

    def pp(x, cols):
        return np.ascontiguousarray(np.asarray(x, f32).reshape(cols, P).T)

    def rep(x):
        x = np.asarray(x, f32)
        return np.ascontiguousarray(np.broadcast_to(x[None, :], (P, len(x))))

    i = {k: np.asarray(v) for k, v in inputs.items()}
    qs, qt = i["sa_bqkv"], i["ta_bqkv"]
    phys = np.stack([i["e_emb"][0, 0], i["m_emb"][0, 0], i["p_emb"][0, 0]])

    shared = dict(
        w_saqkv=wT(i["sa_wqkv"]), w_sao=wT(i["sa_wo"]),
        w_taqkv=wT(i["ta_wqkv"]), w_tao=wT(i["ta_wo"]),
        w_c2b=wT(i["c2b_w"]), w_l2b=wT(i["l2b_w"]),
        w_pq=wT(i["pq_w"]), w_pk=wT(i["pk_w"]), w_pv=wT(i["pv_w"]),
        w_po=wT(i["po_w"]), w_c2l=wT(i["c2l_w"]), w_l2c=wT(i["l2c_w"]),
        physT=np.ascontiguousarray(phys.astype(f32).T).astype(bf16),
        b_saq=pp(qs[0:512], 4), b_sak=pp(qs[512:1024], 4),
        b_sav=rep(qs[1024:1536]).astype(bf16), b_sao=pp(i["sa_bo"], 4),
        b_taq=pp(qt[0:512], 4), b_tak=pp(qt[512:1024], 4),
        b_tav=rep(qt[1024:1536]).astype(bf16), b_tao=pp(i["ta_bo"], 4),
        b_c2b=pp(i["c2b_b"], 8), b_l2b=pp(i["l2b_b"], 8),
        b_pq=pp(i["pq_b"], 8), b_pk=pp(i["pk_b"], 8),
        b_pv=rep(i["pv_b"]).astype(bf16), b_po=pp(i["po_b"], 8),
        b_c2l=pp(i["c2l_b"], 6), b_l2c=pp(i["l2c_b"], 4),
        g_c2b=pp(i["c2b_g"], 8), e_c2b=pp(i["c2b_be"], 8),
        g_l2b=pp(i["l2b_g"], 8), e_l2b=pp(i["l2b_be"], 8),
        g_pa=pp(i["pa_g"], 8), e_pa=pp(i["pa_be"], 8),
        g_c2l=pp(i["c2l_g"], 6), e_c2l=pp(i["c2l_be"], 6),
        g_l2c=pp(i["l2c_g"], 4), e_l2c=pp(i["l2c_be"], 4),
    )
    in_maps = []
    for c in range(N_CORES):
        b, hh = c // 2, c % 2
        m = dict(shared)
        m["xc"] = wT(i["enhanced_cnn_features"][b, hh * R:(hh + 1) * R])
        m["xl"] = wT(i["llm_features"][b, hh * R:(hh + 1) * R])
        in_maps.append(m)
    return in_maps


def kernel(**inputs):
    nc = _get_nc()
    in_maps = _pack_host(inputs)
    res = run_bass_kernel_spmd(nc, in_maps, list(range(N_CORES)))
    cnn = np.empty((B, S, CNN_D), np.float32)
    llm = np.empty((B, S, LLM_D), np.float32)
    for c in range(N_CORES):
        b, hh = c // 2, c % 2
        cnn[b, hh * R:(hh + 1) * R] = res.results[c]["out_cnn"].T
        llm[b, hh * R:(hh + 1) * R] = res.results[c]["out_llm"].T
    return (cnn, llm)


# revision 30
# speedup vs baseline: 1.3611x; 1.0029x over previous
"""Trainium2 Bass kernel for nn_DatacubeLLMBridge (dense_transformer).

Sharding: 8 cores = 4 batches x 2 sequence-halves. Core c owns batch c//2,
rows [(c%2)*512, (c%2+1)*512) of S=1024. Weights replicated. Cross-core
traffic: AllGather of K/V within pairs [[0,1],[2,3],[4,5],[6,7]] (4 total).

Layout: activations are FEATURE-major in SBUF: x^T = [feat chunks of 128
partitions, rows in free dim]. Matmuls contract the partition dim. Weights
host-pretransposed to W^T=[fin,fout] bf16. Row-major V (for attention AV)
is produced by swapping matmul operands (lhsT = x^T row-chunk).

Softmax: the reference's energy/mass biases are constant per query row and
softmax is shift-invariant per row, so they cancel exactly. Score
magnitudes are O(1) here (0.02-scale weights), so exp without
max-subtraction is safe. Softmax denominators come from a ones-augmented V
column; the normalization is folded into the AV-PSUM evict.

PSUM budget (8 banks): dense [128,512]x2 + scores [128,4,512]x1 + avt
[65,512]x2 = 8.
"""

from contextlib import ExitStack

import numpy as np
import ml_dtypes

import concourse.bass as bass
import concourse.mybir as mybir
import concourse.tile as tile
from concourse import bacc
from concourse.bass_utils import run_bass_kernel_spmd

F32 = mybir.dt.float32
BF16 = mybir.dt.bfloat16
AF = mybir.ActivationFunctionType
OP = mybir.AluOpType


# Unify Exp/Ln into one ACT table set (natural_log_exp_and_others) so the
# softmax-reciprocal ln/exp does not thrash ACT_TABLE_LOADs. Contents-only
# edit; set order (act_func_set_id indices) is preserved.
import functools as _ft
import concourse.hw_specs as _hw
import concourse.bacc as _bacc_mod

_ORIG_GAT = _hw.get_activation_tables


@_ft.cache
def _patched_gat(arch):
    t = dict(_ORIG_GAT(arch))
    drop = {AF.Exp, AF.Ln}
    for name in ("exp_and_others", "natural_log"):
        if name in t:
            t[name] = set(t[name]) - drop
    return t


_hw.get_activation_tables = _patched_gat
_bacc_mod.get_activation_tables = _patched_gat

P = 128
B, S, CNN_D, LLM_D, BR_D, H = 4, 1024, 512, 768, 1024, 16
R = 512            # rows per core
N_CORES = 8
GROUPS = [[0, 1], [2, 3], [4, 5], [6, 7]]
EPS = 1e-5
NK = S + 3         # phys attention keys

bf16 = ml_dtypes.bfloat16


class Ker:
    """Holds build state so helpers can share pools/constants."""

    def __init__(self, nc, tc, d):
        self.nc, self.tc, self.d = nc, tc, d
        self.rr_recip = True

    # ------------------------------------------------------------ helpers
    def linear_fm(self, out_sb, wT, x_sb, bias, kch, mch, func=AF.Copy,
                  wcol0=0, uid=""):
        """out^T[:,m,:] = func(sum_k wT[:,k,col].T @ x[:,k,:] + bias[:,m])."""
        nc = self.nc
        ncols = x_sb.shape[-1]
        for m in range(mch):
            ps = self.pps.tile([P, 512], F32, tag="dense", bufs=2,
                               name=f"psd_{uid}_{m}")
            for k in range(kch):
                nc.tensor.matmul(
                    ps[:, :ncols],
                    lhsT=wT[:, k, wcol0 + 128 * m: wcol0 + 128 * (m + 1)],
                    rhs=x_sb[:, k, :],
                    start=(k == 0), stop=(k == kch - 1))
            if func == AF.Copy:
                nc.vector.tensor_scalar(out_sb[:, m, :], ps[:, :ncols],
                                        scalar1=bias[:, m:m + 1],
                                        scalar2=None, op0=OP.add)
            else:
                nc.scalar.activation(out_sb[:, m, :], ps[:, :ncols], func,
                                     bias=bias[:, m:m + 1])

    def vproj_rm(self, out_sb, wT, x_sb, bias_rep, kch, rch, fout,
                 wcol0=0, uid=""):
        """Row-major projection: out[:,rc,:] = x-rows @ W^T + bias."""
        nc = self.nc
        for rc in range(rch):
            for nn in range((fout + 511) // 512):
                n0, n1 = nn * 512, min(fout, (nn + 1) * 512)
                ps = self.pps.tile([P, 512], F32, tag="dense", bufs=2,
                                   name=f"psv_{uid}_{rc}_{nn}")
                for k in range(kch):
                    nc.tensor.matmul(
                        ps[:, : n1 - n0],
                        lhsT=x_sb[:, k, 128 * rc: 128 * (rc + 1)],
                        rhs=wT[:, k, wcol0 + n0: wcol0 + n1],
                        start=(k == 0), stop=(k == kch - 1))
                nc.vector.tensor_add(out_sb[:, rc, n0:n1], ps[:, : n1 - n0],
                                     bias_rep[:, n0:n1])

    def layernorm_fm(self, out_sb, in_sb, gamma, beta, mch, D, uid,
                     out_fn=None):
        """LN over the feature (partition-chunk) dim; in_sb [128,mch,512].
        If out_fn is given it must return the destination AP for chunk m."""
        nc, sp = self.nc, self.spln
        sq = sp.tile([P, 8, 512], BF16, tag="ln_sq", name=f"lnsq_{uid}")
        nc.vector.tensor_mul(sq[:, :mch, :], in_sb[:, :mch, :],
                             in_sb[:, :mch, :])
        ps_s = self.pps.tile([1, 512], F32, tag="dense", bufs=2,
                             name=f"lns_{uid}")
        ps_q = self.pps.tile([1, 512], F32, tag="dense", bufs=2,
                             name=f"lnq_{uid}")
        for k in range(mch):
            nc.tensor.matmul(ps_s[:], lhsT=self.ones_bf[:, 0:1],
                             rhs=in_sb[:, k, :],
                             start=(k == 0), stop=(k == mch - 1))
        for k in range(mch):
            nc.tensor.matmul(ps_q[:], lhsT=self.ones_bf[:, 0:1],
                             rhs=sq[:, k, :],
                             start=(k == 0), stop=(k == mch - 1))
        sm = sp.tile([1, 2, 512], F32, tag="ln_sm", name=f"lnsm_{uid}")
        mean, msq = sm[0:1, 0, :], sm[0:1, 1, :]
        rstd = sp.tile([1, 512], F32, tag="ln_rstd", name=f"lnrs_{uid}")
        mr = sp.tile([1, 512], F32, tag="ln_mr", name=f"lnmr_{uid}")
        nc.vector.tensor_scalar_mul(mean, ps_s[:], 1.0 / D)
        nc.vector.tensor_scalar_mul(msq, ps_q[:], 1.0 / D)
        nc.vector.tensor_mul(rstd[:], mean, mean)      # rstd = mean^2 (tmp)
        nc.vector.tensor_sub(msq, msq, rstd[:])        # msq  = var
        nc.scalar.activation(msq, msq, AF.Sqrt, bias=self.eps_t[0:1, :])
        nc.vector.reciprocal(rstd[:], msq)
        nc.vector.tensor_mul(mr[:], mean, rstd[:])
        bc = sp.tile([P, 2, 512], BF16, tag="ln_bc", name=f"lnbc_{uid}")
        for j, src in ((0, rstd[:]), (1, mr[:])):
            ps_b = self.pps.tile([P, 512], F32, tag="dense", bufs=2,
                                 name=f"lnb_{uid}_{j}")
            nc.tensor.matmul(ps_b[:], lhsT=self.ones_1x128[:], rhs=src,
                             start=True, stop=True)
            nc.vector.tensor_copy(bc[:, j, :], ps_b[:])
        for m in range(mch):
            t = sp.tile([P, 512], BF16, tag="ln_t", bufs=1,
                        name=f"lnt_{uid}_{m}")
            nc.vector.tensor_mul(t[:], in_sb[:, m, :], bc[:, 0, :])
            nc.vector.tensor_sub(t[:], t[:], bc[:, 1, :])
            dst = out_fn(m) if out_fn is not None else out_sb[:, m, :]
            nc.vector.tensor_scalar(dst, t[:],
                                    scalar1=gamma[:, m:m + 1],
                                    scalar2=beta[:, m:m + 1],
                                    op0=OP.mult, op1=OP.add)

    def attention(self, exp_pool, exp_bufs, qT, kT_full, v_packed, attnoutT,
                  dh, n_keys, uid):
        """Phase-separated: all scores+exp for a (group, pair), then AV,
        then the normalize chain. Softmax reciprocal alternates DVE <->
        ACT(ln/exp); ACT table sets unified via get_activation_tables
        patch."""
        nc, sp = self.nc, self.spat
        E = qT.shape[1] * P
        hpc = P // dh
        kc_full, rag = n_keys // P, n_keys % P
        kcN = kc_full + (1 if rag else 0)
        scale = 1.0 / float(np.sqrt(dh))
        npair = hpc // 2
        for g in range(E // P):
            exps = [exp_pool.tile([P, 9, 2, 512], BF16, tag="expS",
                                  bufs=exp_bufs,
                                  name=f"exp_{uid}_{g}_{a}")
                    for a in range(npair)]
            for kc in range(kcN):
                kk = P if kc < kc_full else rag
                for a in range(npair):
                    ps_s = self.pps.tile([P, 2, 512], F32, tag="sc", bufs=2,
                                         name=f"sc_{uid}_{g}_{kc}_{a}")
                    for i in range(2):
                        ho = (2 * a + i) * dh
                        nc.tensor.matmul(
                            ps_s[:kk, i, :],
                            lhsT=kT_full[ho:ho + dh, g, kc * P: kc * P + kk],
                            rhs=qT[ho:ho + dh, g, :],
                            start=True, stop=True,
                            tile_position=(ho, 0))
                    nc.scalar.activation(
                        exps[a][:kk, kc, :, :], ps_s[:kk, :, :],
                        AF.Exp, scale=scale)
            for a in range(npair):
                ps_avs = []
                rps = []
                for i in range(2):
                    h = g * hpc + 2 * a + i
                    ps_av = self.pps.tile([65, 512], F32, tag="avt", bufs=2,
                                          name=f"av_{uid}_{g}_{a}_{i}")
                    for kc in range(kcN):
                        kk = P if kc < kc_full else rag
                        nc.tensor.matmul(
                            ps_av[: dh + 1, :],
                            lhsT=v_packed[:kk, kc, h, :],
                            rhs=exps[a][:kk, kc, i, :],
                            start=(kc == 0), stop=(kc == kcN - 1))
                    rp = sp.tile([1, 512], F32, tag="recip", bufs=2,
                                 name=f"rp_{uid}_{g}_{a}_{i}")
                    if self.rr_recip:
                        nc.vector.reciprocal(rp[:], ps_av[dh:dh + 1, :])
                    else:
                        nc.scalar.activation(rp[:], ps_av[dh:dh + 1, :],
                                             AF.Ln)
                        nc.scalar.activation(rp[:], rp[:], AF.Exp,
                                             scale=-1.0)
                    self.rr_recip = not self.rr_recip
                    ps_avs.append(ps_av)
                    rps.append(rp)
                ps_bc = self.pps.tile([P, 512], F32, tag="dense", bufs=2,
                                      name=f"bc_{uid}_{g}_{a}")
                for i in range(2):
                    ho = (2 * a + i) * dh
                    nc.tensor.matmul(ps_bc[ho:ho + dh, :],
                                     lhsT=self.ones_1x128[0:1, 0:dh],
                                     rhs=rps[i][:],
                                     start=True, stop=True,
                                     tile_position=(0, ho))
                bo0 = 2 * a * dh
                bc_sb = sp.tile([P, 512], BF16, tag="bc_sb", bufs=2,
                                name=f"bcs_{uid}_{g}_{a}")
                nc.scalar.activation(bc_sb[bo0:bo0 + 2 * dh, :],
                                     ps_bc[bo0:bo0 + 2 * dh, :], AF.Copy)
                for i in range(2):
                    ho = (2 * a + i) * dh
                    nc.vector.tensor_tensor(
                        attnoutT[ho:ho + dh, g, :], ps_avs[i][0:dh, :],
                        bc_sb[ho:ho + dh, :], op=OP.mult)


def _build_body(K, ctx):
    nc, tc, d = K.nc, K.tc, K.d

    # ---------------- persistent pools
    pc = ctx.enter_context(tc.tile_pool(name="consts", bufs=1))
    st = ctx.enter_context(tc.tile_pool(name="stream", bufs=1))
    dram = ctx.enter_context(tc.tile_pool(name="drampool", bufs=1,
                                          space="DRAM"))
    K.pps = ctx.enter_context(tc.tile_pool(name="pspool", bufs=1,
                                           space="PSUM"))
    K.spln = ctx.enter_context(tc.tile_pool(name="sp_ln", bufs=1))
    K.spat = ctx.enter_context(tc.tile_pool(name="sp_att", bufs=1))
    spq = ctx.enter_context(tc.tile_pool(name="sp_q", bufs=1))

    def load_pp(name, cols, dt=F32):
        t = pc.tile([P, cols], dt, name=f"c_{name}")
        nc.sync.dma_start(t[:], d[name][:])
        return t

    bias = {n: load_pp(n, c) for n, c in [
        ("b_saq", 4), ("b_sak", 4), ("b_sao", 4), ("b_taq", 4), ("b_tak", 4),
        ("b_tao", 4), ("b_c2b", 8), ("b_l2b", 8), ("b_pq", 8), ("b_pk", 8),
        ("b_po", 8), ("b_c2l", 6), ("b_l2c", 4), ("g_c2b", 8), ("e_c2b", 8),
        ("g_l2b", 8), ("e_l2b", 8), ("g_pa", 8), ("e_pa", 8), ("g_c2l", 6),
        ("e_c2l", 6), ("g_l2c", 4), ("e_l2c", 4)]}
    for n, c in [("b_sav", CNN_D), ("b_tav", CNN_D), ("b_pv", BR_D)]:
        bias[n] = load_pp(n, c, dt=BF16)

    K.ones_bf = pc.tile([P, 1], BF16, name="ones_bf")
    nc.vector.memset(K.ones_bf[:], 1.0)
    K.ones_1x128 = pc.tile([1, P], F32, name="ones_1x128")
    nc.vector.memset(K.ones_1x128[:], 1.0)
    K.ones_1x128_bf = pc.tile([1, P], BF16, name="ones_1x128_bf")
    nc.vector.memset(K.ones_1x128_bf[:], 1.0)
    K.eps_t = pc.tile([1, 1], F32, name="eps_t")
    nc.vector.memset(K.eps_t[:], EPS)
    physT = pc.tile([P, 8, 3], BF16, name="physT_sb")
    nc.sync.dma_start(physT[:], d["physT"].rearrange("(c p) t -> p c t", p=P))

    def load_w(pool, name, kch, fout, tag=None, bufs=1):
        t = pool.tile([P, kch, fout], BF16, tag=tag or f"w{name}", bufs=bufs,
                      name=f"w_{name}")
        # gpsimd DMA queue: a weight load that waits for SBUF space must
        # not head-of-line-block the sync queue (bounces/gathers).
        nc.gpsimd.dma_start(t[:],
                            d[f"w_{name}"].rearrange("(c p) n -> p c n", p=P))
        return t

    aug_l = st.tile([P, 8, R], BF16, tag="aug", bufs=2, name="aug_l")
    aug_c = st.tile([P, 8, R], BF16, tag="aug", bufs=2, name="aug_c")
    q_c = spq.tile([P, 8, R], BF16, tag="physq", bufs=2, name="q_c")
    q_l = spq.tile([P, 8, R], BF16, tag="physq", bufs=2, name="q_l")
    kT_phys = spq.tile([P, 8, 3], BF16, name="kT_phys")
    v_phys = spq.tile([3, BR_D], BF16, name="v_phys")

    # ======================================================= front stages
    with tc.tile_pool(name="front", bufs=1) as front, \
         tc.tile_pool(name="w_c", bufs=1) as wC, \
         tc.tile_pool(name="sp_br", bufs=1) as spbr:

        x0 = front.tile([P, 4, R], BF16, tag="xs", bufs=2, name="x0")
        nc.sync.dma_start(x0[:], d["xc"].rearrange("(c p) j -> p c j", p=P))
        xl = front.tile([P, 6, R], BF16, name="xl")
        nc.sync.dma_start(xl[:], d["xl"].rearrange("(c p) j -> p c j", p=P))

        def bridge(x_sb, wt, bname, mch, D, out_sb, uid, kch):
            g = spbr.tile([P, 8, R], BF16, tag="gelu", bufs=1,
                          name=f"gelu_{uid}")
            K.linear_fm(g, wt, x_sb, bias[f"b_{bname}"], kch, mch,
                        func=AF.Gelu, uid=f"br_{uid}")
            K.layernorm_fm(out_sb, g, bias[f"g_{bname}"], bias[f"e_{bname}"],
                           mch, D, uid)

        with tc.tile_pool(name="w_ab", bufs=1) as wAB, \
             tc.tile_pool(name="sp_mha", bufs=1) as spm:

            def mha_front(x_sb, wqkv, bq, bk, bv_rep, mid):
                qT = spm.tile([P, 4, R], BF16, tag="qT", name=f"qT_{mid}")
                kTl = spm.tile([P, 4, R], BF16, tag="kTl", name=f"kTl_{mid}")
                vl = spm.tile([P, 4, CNN_D], BF16, tag="vl", name=f"vl_{mid}")
                K.linear_fm(qT, wqkv, x_sb, bq, 4, 4, wcol0=0, uid=f"q{mid}")
                K.linear_fm(kTl, wqkv, x_sb, bk, 4, 4, wcol0=512,
                            uid=f"k{mid}")
                K.vproj_rm(vl, wqkv, x_sb, bv_rep, 4, 4, CNN_D, wcol0=1024,
                           uid=f"v{mid}")
                kv_loc = dram.tile([2 * CNN_D, R], BF16, name=f"kvl_{mid}")
                kv_full = dram.tile([4 * CNN_D, R], BF16, name=f"kvf_{mid}")
                nc.sync.dma_start(
                    kv_loc[0:512, :].rearrange("(c p) j -> p c j", p=P),
                    kTl[:])
                nc.sync.dma_start(
                    kv_loc[512:1024, :].rearrange("(c p) j -> p c j", p=P),
                    vl[:])
                nc.gpsimd.collective_compute(
                    "AllGather", OP.bypass, replica_groups=GROUPS,
                    ins=[kv_loc.opt()], outs=[kv_full.opt()])
                return qT, kv_full

            def mha_attn(x_sb, qT, kv_full, wo, bo, mid):
                kT = spm.tile([P, 4, S], BF16, tag="kT", name=f"kT_{mid}")
                vpk = spm.tile([P, 8, H, 33], BF16, tag="vpk",
                               name=f"vpk_{mid}")
                nc.vector.memset(vpk[:, :, :, 32:33], 1.0)
                for r in range(2):
                    nc.sync.dma_start(
                        kT[:, :, 512 * r: 512 * (r + 1)],
                        kv_full[1024 * r: 1024 * r + 512, :]
                        .rearrange("(c p) j -> p c j", p=P))
                    vpl = spm.tile([P, 4, CNN_D], BF16, tag="vpl", bufs=1,
                                   name=f"vpl_{mid}_{r}")
                    nc.sync.dma_start(
                        vpl[:],
                        kv_full[1024 * r + 512: 1024 * (r + 1), :]
                        .rearrange("(c p) f -> p c f", p=P))
                    for h in range(H):
                        nc.vector.tensor_copy(
                            vpk[:, 4 * r: 4 * r + 4, h, 0:32],
                            vpl[:, :, 32 * h: 32 * h + 32])
                attnT = spm.tile([P, 4, R], BF16, tag="attnT",
                                 name=f"at_{mid}")
                K.attention(spm, 2, qT, kT, vpk, attnT, 32, S, mid)
                xo = front.tile([P, 4, R], BF16, tag="xs", bufs=2,
                                name=f"x_{mid}")
                for m in range(4):
                    ps = K.pps.tile([P, 512], F32, tag="dense", bufs=2,
                                    name=f"pso_{mid}_{m}")
                    for k in range(4):
                        nc.tensor.matmul(
                            ps[:], lhsT=wo[:, k, 128 * m:128 * (m + 1)],
                            rhs=attnT[:, k, :], start=(k == 0), stop=(k == 3))
                    t = spm.tile([P, 512], BF16, tag="otmp", bufs=1,
                                 name=f"ot_{mid}_{m}")
                    nc.vector.tensor_scalar(t[:], ps[:],
                                            scalar1=bo[:, m:m + 1],
                                            scalar2=None, op0=OP.add)
                    nc.vector.tensor_add(xo[:, m, :], t[:], x_sb[:, m, :])
                return xo

            # ---- MHA1 (l2b bridge overlaps the CC1 gather)
            w_saqkv = load_w(wAB, "saqkv", 4, 1536, tag="wqkv", bufs=1)
            w_sao = load_w(wAB, "sao", 4, 512, tag="wsq", bufs=1)
            qA, kvfA = mha_front(x0, w_saqkv, bias["b_saq"], bias["b_sak"],
                                 bias["b_sav"], "m1")
            w_l2b = load_w(wC, "l2b", 6, BR_D, tag="wc", bufs=1)
            bridge(xl, w_l2b, "l2b", 8, BR_D, aug_l, "l2b", kch=6)
            x1 = mha_attn(x0, qA, kvfA, w_sao, bias["b_sao"], "m1")

            # ---- MHA2
            w_taqkv = load_w(wAB, "taqkv", 4, 1536, tag="wqkv", bufs=1)
            w_tao = load_w(wAB, "tao", 4, 512, tag="wsq", bufs=1)
            qB, kvfB = mha_front(x1, w_taqkv, bias["b_taq"], bias["b_tak"],
                                 bias["b_tav"], "m2")
            x2 = mha_attn(x1, qB, kvfB, w_tao, bias["b_tao"], "m2")

        # ================================== phys projections + c2b bridge
        with tc.tile_pool(name="w_d", bufs=1) as wD, \
             tc.tile_pool(name="sp_d", bufs=1) as spD:
            w_pk = load_w(wD, "pk", 8, BR_D, tag="wbig", bufs=2)
            w_pv = load_w(wD, "pv", 8, BR_D, tag="wbig", bufs=2)

            def phys_kv(aug, sid):
                kTl = spD.tile([P, 8, R], BF16, tag="pkTl", bufs=1,
                               name=f"pkTl_{sid}")
                vl = spD.tile([P, 4, BR_D], BF16, tag="pvl", bufs=1,
                              name=f"pvl_{sid}")
                K.linear_fm(kTl, w_pk, aug, bias["b_pk"], 8, 8,
                            uid=f"pk{sid}")
                K.vproj_rm(vl, w_pv, aug, bias["b_pv"], 8, 4, BR_D,
                           uid=f"pv{sid}")
                loc = dram.tile([2048, 512], BF16, name=f"pb_{sid}")
                full = dram.tile([4096, 512], BF16, name=f"pf_{sid}")
                nc.sync.dma_start(
                    loc[0:1024, :].rearrange("(c p) j -> p c j", p=P), kTl[:])
                nc.sync.dma_start(
                    loc[1024:2048, :].rearrange("(c p u) j -> p c u j",
                                                p=P, u=2),
                    vl[:].rearrange("p c (u j) -> p c u j", u=2))
                nc.gpsimd.collective_compute(
                    "AllGather", OP.bypass, replica_groups=GROUPS,
                    ins=[loc.opt()], outs=[full.opt()])
                return full

            pf_l = phys_kv(aug_l, "l")

            # c2b bridge
            w_c2b = load_w(wC, "c2b", 4, BR_D, tag="wc", bufs=1)
            bridge(x2, w_c2b, "c2b", 8, BR_D, aug_c, "c2b", kch=4)

            pf_c = phys_kv(aug_c, "c")

            # phys-token K/V (local; identical on both pair members)
            for m in range(8):
                ps = K.pps.tile([P, 512], F32, tag="dense", bufs=2,
                                name=f"pspk_{m}")
                for k in range(8):
                    nc.tensor.matmul(ps[:, 0:3],
                                     lhsT=w_pk[:, k, 128 * m:128 * (m + 1)],
                                     rhs=physT[:, k, :],
                                     start=(k == 0), stop=(k == 7))
                nc.vector.tensor_scalar(kT_phys[:, m, :], ps[:, 0:3],
                                        scalar1=bias["b_pk"][:, m:m + 1],
                                        scalar2=None, op0=OP.add)
            for nn in range(2):
                ps = K.pps.tile([P, 512], F32, tag="dense", bufs=2,
                                name=f"pspv_{nn}")
                for k in range(8):
                    nc.tensor.matmul(ps[0:3, :], lhsT=physT[:, k, :],
                                     rhs=w_pv[:, k, 512 * nn: 512 * (nn + 1)],
                                     start=(k == 0), stop=(k == 7))
                nc.vector.tensor_add(
                    v_phys[:, 512 * nn:512 * (nn + 1)], ps[0:3, :],
                    bias["b_pv"][0:3, 512 * nn:512 * (nn + 1)])

            w_pq = load_w(wD, "pq", 8, BR_D, tag="wbig", bufs=2)
            K.linear_fm(q_c, w_pq, aug_c, bias["b_pq"], 8, 8, uid="qc")
            K.linear_fm(q_l, w_pq, aug_l, bias["b_pq"], 8, 8, uid="ql")

    # ======================================================== phys attns
    with tc.tile_pool(name="w_e", bufs=1) as wE, \
         tc.tile_pool(name="sp_e", bufs=1) as spE:
        w_po = load_w(wE, "po", 8, BR_D)

        def phys_gather_in(full, sid):
            kT = spE.tile([P, 8, NK], BF16, tag="physkT", bufs=1,
                          name=f"kTf_{sid}")
            vpl = spE.tile([P, 8, BR_D], BF16, tag="physvpl", bufs=1,
                           name=f"vplf_{sid}")
            for r in range(2):
                nc.sync.dma_start(
                    kT[:, :, 512 * r: 512 * (r + 1)],
                    full[2048 * r: 2048 * r + 1024, :]
                    .rearrange("(c p) j -> p c j", p=P))
                nc.sync.dma_start(
                    vpl[:, 4 * r: 4 * (r + 1), :]
                    .rearrange("p c (u j) -> p c u j", u=2),
                    full[2048 * r + 1024: 2048 * (r + 1), :]
                    .rearrange("(c p u) j -> p c u j", p=P, u=2))
            nc.vector.tensor_copy(kT[:, :, 1024:1027], kT_phys[:])
            vpk = spE.tile([P, 9, H, 65], BF16, tag="physvpk", bufs=1,
                           name=f"vpk_{sid}")
            nc.vector.memset(vpk[:, :, :, 64:65], 1.0)
            for h in range(H):
                nc.vector.tensor_copy(vpk[:, 0:8, h, 0:64],
                                      vpl[:, :, 64 * h: 64 * h + 64])
                nc.vector.tensor_copy(vpk[0:3, 8, h, 0:64],
                                      v_phys[:, 64 * h: 64 * h + 64])
            return kT, vpk

        def phys_attn(qT, kT, vpk, residual, out_res, sid):
            attnT = spE.tile([P, 8, R], BF16, tag="pattnT", bufs=2,
                             name=f"pat_{sid}")
            K.attention(spE, 1, qT, kT, vpk, attnT, 64, NK, f"p{sid}")
            pre = spE.tile([P, 8, R], BF16, tag="pattnT", bufs=2,
                           name=f"pre_{sid}")
            for m in range(8):
                ps = K.pps.tile([P, 512], F32, tag="dense", bufs=2,
                                name=f"pso_{sid}_{m}")
                for k in range(8):
                    nc.tensor.matmul(
                        ps[:], lhsT=w_po[:, k, 128 * m:128 * (m + 1)],
                        rhs=attnT[:, k, :], start=(k == 0), stop=(k == 7))
                t = spE.tile([P, 512], BF16, tag="potmp", bufs=1,
                             name=f"pot_{sid}_{m}")
                nc.vector.tensor_scalar(t[:], ps[:],
                                        scalar1=bias["b_po"][:, m:m + 1],
                                        scalar2=None, op0=OP.add)
                nc.vector.tensor_add(pre[:, m, :], t[:], residual[:, m, :])
            K.layernorm_fm(out_res, pre, bias["g_pa"], bias["e_pa"], 8,
                           BR_D, f"pa_{sid}")

        def final_bridge(x_sb, wname, mch, D, out_d, uid):
            wF = load_w(wE, wname, 8, D, tag="wF", bufs=1)
            gf = spE.tile([P, 8, R], BF16, tag="gF", bufs=1,
                          name=f"gF_{uid}")
            K.linear_fm(gf, wF, x_sb, bias[f"b_{wname}"], 8, mch,
                        func=AF.Gelu, uid=f"fb_{uid}")
            out_r = out_d.rearrange("(c p) j -> p c j", p=P)

            def out_fn(m):
                t = spE.tile([P, 512], F32, tag="oF", bufs=2,
                             name=f"oFc_{uid}_{m}")
                out_fn.pending.append((m, t))
                return t[:]
            out_fn.pending = []
            K.layernorm_fm(None, gf, bias[f"g_{wname}"], bias[f"e_{wname}"],
                           mch, D, f"f_{uid}", out_fn=out_fn)
            for m, t in out_fn.pending:
                nc.sync.dma_start(out_r[:, m, :], t[:])

        kT_l, vpk_l = phys_gather_in(pf_l, "l")
        cnn_att = spE.tile([P, 8, R], BF16, tag="attres", bufs=1,
                           name="cnn_att")
        phys_attn(q_c, kT_l, vpk_l, aug_c, cnn_att, "c")
        final_bridge(cnn_att, "c2l", 6, LLM_D, d["out_llm"], "c2l")

        kT_c, vpk_c = phys_gather_in(pf_c, "c")
        llm_att = spE.tile([P, 8, R], BF16, tag="attres", bufs=1,
                           name="llm_att")
        phys_attn(q_l, kT_c, vpk_c, aug_l, llm_att, "l")
        final_bridge(llm_att, "l2c", 4, CNN_D, d["out_cnn"], "l2c")


def build_kernel():
    nc = bacc.Bacc("TRN2", target_bir_lowering=False, debug=False,
                   num_devices=N_CORES)

    def din(name, shape, dt=BF16):
        return nc.dram_tensor(name, shape, dt, kind="ExternalInput")

    d = {}
    d["xc"] = din("xc", [CNN_D, R])
    d["xl"] = din("xl", [LLM_D, R])
    for n, sh in [("saqkv", [CNN_D, 3 * CNN_D]), ("sao", [CNN_D, CNN_D]),
                  ("taqkv", [CNN_D, 3 * CNN_D]), ("tao", [CNN_D, CNN_D]),
                  ("c2b", [CNN_D, BR_D]), ("l2b", [LLM_D, BR_D]),
                  ("pq", [BR_D, BR_D]), ("pk", [BR_D, BR_D]),
                  ("pv", [BR_D, BR_D]), ("po", [BR_D, BR_D]),
                  ("c2l", [BR_D, LLM_D]), ("l2c", [BR_D, CNN_D])]:
        d[f"w_{n}"] = din(f"w_{n}", sh)
    d["physT"] = din("physT", [BR_D, 3])
    for n, c in [("b_saq", 4), ("b_sak", 4), ("b_sao", 4), ("b_taq", 4),
                 ("b_tak", 4), ("b_tao", 4), ("b_c2b", 8), ("b_l2b", 8),
                 ("b_pq", 8), ("b_pk", 8), ("b_po", 8), ("b_c2l", 6),
                 ("b_l2c", 4), ("g_c2b", 8), ("e_c2b", 8), ("g_l2b", 8),
                 ("e_l2b", 8), ("g_pa", 8), ("e_pa", 8), ("g_c2l", 6),
                 ("e_c2l", 6), ("g_l2c", 4), ("e_l2c", 4),
                 ]:
        d[n] = din(n, [P, c], F32)
    for n, c in [("b_sav", CNN_D), ("b_tav", CNN_D), ("b_pv", BR_D)]:
        d[n] = din(n, [P, c], BF16)
    d["out_cnn"] = nc.dram_tensor("out_cnn", [CNN_D, R], F32,
                                  kind="ExternalOutput")
    d["out_llm"] = nc.dram_tensor("out_llm", [LLM_D, R], F32,
                                  kind="ExternalOutput")

    with tile.TileContext(nc) as tc:
        with ExitStack() as ctx:
            K = Ker(nc, tc, d)
            _build_body(K, ctx)
    nc.compile()
    return nc


# ------------------------------------------------------------------- host

_NC_CACHE = None


def _get_nc():
    global _NC_CACHE
    if _NC_CACHE is None:
        _NC_CACHE = build_kernel()
    return _NC_CACHE


def _pack_host(inputs):
    f32 = np.float32

    def wT(x):
        return np.ascontiguousarray(np.asarray(x, f32).T).astype(bf16)

    def pp(x, cols):
        return np.ascontiguousarray(np.asarray(x, f32).reshape(cols, P).T)

    def rep(x):
        x = np.asarray(x, f32)
        return np.ascontiguousarray(np.broadcast_to(x[None, :], (P, len(x))))

    i = {k: np.asarray(v) for k, v in inputs.items()}
    qs, qt = i["sa_bqkv"], i["ta_bqkv"]
    phys = np.stack([i["e_emb"][0, 0], i["m_emb"][0, 0], i["p_emb"][0, 0]])

    shared = dict(
        w_saqkv=wT(i["sa_wqkv"]), w_sao=wT(i["sa_wo"]),
        w_taqkv=wT(i["ta_wqkv"]), w_tao=wT(i["ta_wo"]),
        w_c2b=wT(i["c2b_w"]), w_l2b=wT(i["l2b_w"]),
        w_pq=wT(i["pq_w"]), w_pk=wT(i["pk_w"]), w_pv=wT(i["pv_w"]),
        w_po=wT(i["po_w"]), w_c2l=wT(i["c2l_w"]), w_l2c=wT(i["l2c_w"]),
        physT=np.ascontiguousarray(phys.astype(f32).T).astype(bf16),
        b_saq=pp(qs[0:512], 4), b_sak=pp(qs[512:1024], 4),
        b_sav=rep(qs[1024:1536]).astype(bf16), b_sao=pp(i["sa_bo"], 4),
        b_taq=pp(qt[0:512], 4), b_tak=pp(qt[512:1024], 4),
        b_tav=rep(qt[1024:1536]).astype(bf16), b_tao=pp(i["ta_bo"], 4),
        b_c2b=pp(i["c2b_b"], 8), b_l2b=pp(i["l2b_b"], 8),
        b_pq=pp(i["pq_b"], 8), b_pk=pp(i["pk_b"], 8),
        b_pv=rep(i["pv_b"]).astype(bf16), b_po=pp(i["po_b"], 8),
        b_c2l=pp(i["c2l_b"], 6), b_l2c=pp(i["l2c_b"], 4),
        g_c2b=pp(i["c2b_g"], 8), e_c2b=pp(i["c2b_be"], 8),
        g_l2b=pp(i["l2b_g"], 8), e_l2b=pp(i["l2b_be"], 8),
        g_pa=pp(i["pa_g"], 8), e_pa=pp(i["pa_be"], 8),
        g_c2l=pp(i["c2l_g"], 6), e_c2l=pp(i["c2l_be"], 6),
        g_l2c=pp(i["l2c_g"], 4), e_l2c=pp(i["l2c_be"], 4),
    )
    in_maps = []
    for c in range(N_CORES):
        b, hh = c // 2, c % 2
        m = dict(shared)
        m["xc"] = wT(i["enhanced_cnn_features"][b, hh * R:(hh + 1) * R])
        m["xl"] = wT(i["llm_features"][b, hh * R:(hh + 1) * R])
        in_maps.append(m)
    return in_maps


def kernel(**inputs):
    nc = _get_nc()
    in_maps = _pack_host(inputs)
    try:
        res = run_bass_kernel_spmd(nc, in_maps, list(range(N_CORES)))
    except Exception:
        import time
        time.sleep(10)  # transient NRT device errors have been observed
        res = run_bass_kernel_spmd(nc, in_maps, list(range(N_CORES)))
    cnn = np.empty((B, S, CNN_D), np.float32)
    llm = np.empty((B, S, LLM_D), np.float32)
    for c in range(N_CORES):
        b, hh = c // 2, c % 2
        cnn[b, hh * R:(hh + 1) * R] = res.results[c]["out_cnn"].T
        llm[b, hh * R:(hh + 1) * R] = res.results[c]["out_llm"].T
    return (cnn, llm)
